# revision 9
# baseline (speedup 1.0000x reference)
"""Multi-head attention (B=4, S=2048, D=1024, H=16, Dk=64) on 8 trn2 NeuronCores.

Sharding: core = (batch b, head-group g), g selects 8 heads (512 proj cols).
Host sums the two partial out-projections per batch and adds bo.

Key optimizations over the v1 kernel (912us):
  * Host-side key compaction: masked keys give exactly-zero probs in the
    reference (exp(-1e9/8) underflows), so drop them on the host and pad
    k/v to SK=1152 rows (mask is Bernoulli(0.5), so ~1024 survive; fall
    back to SK=2048 if a batch ever exceeds 1152).  Cuts k/v projections,
    scores, attn and the scalar-engine exp work by ~44%.
  * bf16 matmuls everywhere (host pre-casts inputs/weights): same 1
    col/cycle stream rate as f32r but fast weight loads (FWL), half the
    DMA and SBUF footprint.  fp32 accumulation in PSUM.
  * Row-tiled scores: the K=64 scores matmuls of the two heads of a pair
    run concurrently in PE row groups 0/64 (tile_position auto-derived
    from the partition bases) -> 2x PE throughput on scores.
  * Pad-key handling via a per-chunk -1e9 activation bias (probs of pad
    keys are exactly 0), plus an indicator column in vh giving the
    softmax denominator for free (row 64 of the attn accumulator).
  * Normalization via replicate-denominator matmul + reciprocal_approx_fast
    on 64 partitions (the v1 kernel burned 6.5us per [1,1024] serial
    reciprocal).
  * Software-pipelined emission: the q projections for query blocks 1-3
    and the out-projection (phase C) are woven into the ACT-bound
    attention loop as PE filler so the tensor engine never idles (keeps
    the HAM clock gate at 2.4 GHz; the v1 kernel sat at 1.2 GHz for
    600us of its runtime).
"""

import sys

sys.path.insert(0, "/opt/trn_rl_repo")

import numpy as np

B, S, D, H, DK = 4, 2048, 1024, 16, 64
CPG = 512          # projection columns per core (8 heads x 64)
NCORES = 8
SK_FAST = 1152     # compacted+padded key rows (multiple of 128)

_cache = {}


def _build_nc(SK):
    import contextlib
    from collections import deque

    import concourse.bass as bass
    import concourse.tile as tile
    from concourse import bacc, mybir

    f32 = mybir.dt.float32
    R = mybir.dt.float32r
    BF = mybir.dt.bfloat16
    Exp = mybir.ActivationFunctionType.Exp

    NSK = SK // 128        # key chunks of 128
    NQB = S // 512         # query 512-blocks (4)
    NDCH = D // 128        # contraction chunks for projections (8)
    NPAIR = 4              # head pairs per core

    nc = bacc.Bacc("TRN2", target_bir_lowering=False, debug=False)

    q_d = nc.dram_tensor("q", [S, D], BF, kind="ExternalInput").ap()
    k_d = nc.dram_tensor("kc", [SK, D], BF, kind="ExternalInput").ap()
    v_d = nc.dram_tensor("vc", [SK, D], BF, kind="ExternalInput").ap()
    wq_d = nc.dram_tensor("wq", [D, CPG], BF, kind="ExternalInput").ap()
    wk_d = nc.dram_tensor("wk", [D, CPG], BF, kind="ExternalInput").ap()
    wv_d = nc.dram_tensor("wv", [D, CPG], BF, kind="ExternalInput").ap()
    wo_d = nc.dram_tensor("wo", [CPG, D], BF, kind="ExternalInput").ap()
    bq_d = nc.dram_tensor("bq", [CPG], BF, kind="ExternalInput").ap()
    bk_d = nc.dram_tensor("bk", [CPG], BF, kind="ExternalInput").ap()
    bv_d = nc.dram_tensor("bv", [CPG], BF, kind="ExternalInput").ap()
    mb_d = nc.dram_tensor("maskbias", [128, NSK], f32, kind="ExternalInput").ap()
    ki_d = nc.dram_tensor("keyind", [128, NSK * 8], BF, kind="ExternalInput").ap()
    ones_d = nc.dram_tensor("ones", [128, 512], BF, kind="ExternalInput").ap()
    onesr_d = nc.dram_tensor("onesr", [1, 64], R, kind="ExternalInput").ap()
    ident_d = nc.dram_tensor("ident", [128, 128], BF, kind="ExternalInput").ap()
    out_d = nc.dram_tensor("out", [S, D], f32, kind="ExternalOutput").ap()

    with tile.TileContext(nc) as tc:
        import contextlib

        with contextlib.ExitStack() as ctx:
            # ---------- persistent tensors + constants ----------
            persist = ctx.enter_context(tc.tile_pool(name="persist", bufs=1))
            consts = ctx.enter_context(tc.tile_pool(name="consts", bufs=1))

            qhT_sb = persist.tile([128, NPAIR, S], BF)     # [c%128, pair, sq]
            khT_sb = persist.tile([128, NPAIR, SK], BF)
            vh_sb = persist.tile([128, NSK, 8, DK + 1], BF)  # ind col at 64
            concatT_sb = persist.tile([128, NPAIR, S], BF)
            wq_sb = persist.tile([128, NDCH, CPG], BF)
            wk_sb = persist.tile([128, NDCH, CPG], BF)
            wv_sb = persist.tile([128, NDCH, CPG], BF)
            wo_sb = persist.tile([128, NPAIR, D], BF)

            ones_sb = consts.tile([1, 512], BF)
            onesr_sb = consts.tile([1, 64], R)
            ident = consts.tile([128, 128], BF)
            mb_sb = consts.tile([128, NSK], f32)
            bq_sb = consts.tile([1, CPG], BF)
            bk_sb = consts.tile([1, CPG], BF)
            bv_sb = consts.tile([1, CPG], BF)

            # DMA emission is ordered so the PE can start within a few us:
            # first k x-tiles + what the first transposes/projections need;
            # the rest of the weights just-in-time before their consumers.
            def dma_weights(t_sb, t_d):
                for j in range(NDCH):
                    nc.sync.dma_start(
                        out=t_sb[:, j, :], in_=t_d[j * 128 : j * 128 + 128, :]
                    )

            # ---------- shared rings ----------
            # PSUM: sc 2x[128,1024]f32 (4 banks) + at 2x[128,512]f32 (2)
            #       + fill 2x[128,512] (2) = 8 banks exactly.
            psum = ctx.enter_context(tc.tile_pool(name="psum", bufs=2, space="PSUM"))
            natpool = ctx.enter_context(tc.tile_pool(name="natpool", bufs=8))
            xtpool = ctx.enter_context(tc.tile_pool(name="xtpool", bufs=10))
            probpool = ctx.enter_context(tc.tile_pool(name="probpool", bufs=3))
            smallpool = ctx.enter_context(tc.tile_pool(name="smallpool", bufs=4))
            outpool = ctx.enter_context(tc.tile_pool(name="outpool", bufs=3))

            # ---------- projection block emitters ----------
            def proj_block_units(kind, x_d, w_sb, b_sb, s0, w, act_copy,
                                 nats_in=None):
                """Generate unit-closures for projecting x rows [s0, s0+w).
                kind: 'q'/'k' -> [c, s] into qhT_sb/khT_sb; 'v' -> vh_sb."""
                nsub = w // 128
                nats = nats_in if nats_in is not None else []
                xts = []

                def u_load():
                    for i in range(nsub):
                        nat = natpool.tile([128, D], BF, tag="nat")
                        r0 = s0 + i * 128
                        nc.sync.dma_start(out=nat, in_=x_d[r0 : r0 + 128, :])
                        nats.append(nat)

                if nats_in is None:
                    yield 0.1, u_load

                def u_tp(j):
                    def run():
                        tp = psum.tile([128, 512], BF, tag="fill")
                        for i in range(nsub):
                            nc.tensor.transpose(
                                out=tp[:, i * 128 : i * 128 + 128],
                                in_=nats[i][:, j * 128 : j * 128 + 128],
                                identity=ident,
                            )
                        xt = xtpool.tile([128, 512], BF, tag="xt")
                        if act_copy:
                            nc.scalar.copy(out=xt[:, :w], in_=tp[:, :w])
                        else:
                            nc.vector.tensor_copy(out=xt[:, :w], in_=tp[:, :w])
                        xts.append(xt)

                    return run

                for j in range(NDCH):
                    yield 0.3, u_tp(j)

                if kind in ("q", "k"):
                    dst = qhT_sb if kind == "q" else khT_sb

                    def u_proj_a(cch, box):
                        def run():
                            pr = psum.tile([128, 512], f32, tag="fill")
                            box.append(pr)
                            nc.tensor.matmul(
                                pr[:, :w],
                                lhsT=b_sb[0:1, cch * 128 : cch * 128 + 128],
                                rhs=ones_sb[0:1, :w],
                                start=True,
                                stop=False,
                            )
                            for j in range(NDCH // 2):
                                nc.tensor.matmul(
                                    pr[:, :w],
                                    lhsT=w_sb[:, j, cch * 128 : cch * 128 + 128],
                                    rhs=xts[j][:, :w],
                                    start=False,
                                    stop=False,
                                )

                        return run

                    def u_proj_b(cch, box):
                        def run():
                            pr = box.pop()
                            for j in range(NDCH // 2, NDCH):
                                nc.tensor.matmul(
                                    pr[:, :w],
                                    lhsT=w_sb[:, j, cch * 128 : cch * 128 + 128],
                                    rhs=xts[j][:, :w],
                                    start=False,
                                    stop=(j == NDCH - 1),
                                )
                            nc.vector.tensor_copy(
                                out=dst[:, cch, s0 : s0 + w], in_=pr[:, :w]
                            )

                        return run

                    for cch in range(NPAIR):
                        box = []
                        yield 1.0, u_proj_a(cch, box)
                        yield 1.0, u_proj_b(cch, box)
                else:

                    def u_projv_a(sub, box):
                        def run():
                            pr = psum.tile([128, 512], f32, tag="fill")
                            box.append(pr)
                            nc.tensor.matmul(
                                pr,
                                lhsT=ones_sb[0:1, 0:128],
                                rhs=b_sb[0:1, :],
                                start=True,
                                stop=False,
                            )
                            for j in range(NDCH // 2):
                                nc.tensor.matmul(
                                    pr,
                                    lhsT=xts[j][:, sub * 128 : sub * 128 + 128],
                                    rhs=w_sb[:, j, :],
                                    start=False,
                                    stop=False,
                                )

                        return run

                    def u_projv_b(sub, box):
                        def run():
                            pr = box.pop()
                            for j in range(NDCH // 2, NDCH):
                                nc.tensor.matmul(
                                    pr,
                                    lhsT=xts[j][:, sub * 128 : sub * 128 + 128],
                                    rhs=w_sb[:, j, :],
                                    start=False,
                                    stop=(j == NDCH - 1),
                                )
                            skc = (s0 + sub * 128) // 128
                            nc.vector.tensor_copy(
                                out=vh_sb[:, skc, :, 0:DK],
                                in_=pr.rearrange("p (h d) -> p h d", h=8),
                            )

                        return run

                    for sub in range(nsub):
                        box = []
                        yield 1.0, u_projv_a(sub, box)
                        yield 1.0, u_projv_b(sub, box)

            def phasec_units(qc):
                """Out-projection for query block qc (concatT -> out)."""

                def u_cblk(sqc, do):
                    def run():
                        o_ps = psum.tile([128, 512], f32, tag="fill")
                        for p in range(NPAIR):
                            nc.tensor.matmul(
                                o_ps,
                                lhsT=concatT_sb[
                                    :, p, sqc * 128 : sqc * 128 + 128
                                ],
                                rhs=wo_sb[:, p, do * 512 : do * 512 + 512],
                                start=(p == 0),
                                stop=(p == NPAIR - 1),
                            )
                        o_sb = outpool.tile([128, 512], f32, tag="osb")
                        nc.vector.tensor_copy(out=o_sb, in_=o_ps)
                        nc.sync.dma_start(
                            out=out_d[
                                sqc * 128 : sqc * 128 + 128,
                                do * 512 : do * 512 + 512,
                            ],
                            in_=o_sb,
                        )

                    return run

                for sq in range(4):
                    for do in range(2):
                        yield 1.0, u_cblk(qc * 4 + sq, do)

            def drain(units):
                for _, u in units:
                    u()

            # ---------- prefix: k, v and q block 0 (ACT does the copies) ----
            kblocks = []
            o = 0
            while o < SK:
                w = min(512, SK - o)
                kblocks.append((o, w))
                o += w

            # first k x-tiles + immediate deps, then weights just-in-time
            knats0 = []
            for i in range(4):
                nat = natpool.tile([128, D], BF, tag="nat")
                nc.sync.dma_start(out=nat, in_=k_d[i * 128 : i * 128 + 128, :])
                knats0.append(nat)
            nc.sync.dma_start(out=ident, in_=ident_d)
            nc.sync.dma_start(out=ones_sb, in_=ones_d[0:1, :])
            nc.sync.dma_start(out=bk_sb, in_=bk_d[None, :])
            dma_weights(wk_sb, wk_d)
            for s0, w in kblocks:
                drain(
                    proj_block_units(
                        "k", k_d, wk_sb, bk_sb, s0, w, True,
                        nats_in=knats0 if s0 == 0 else None,
                    )
                )
            nc.sync.dma_start(out=bv_sb, in_=bv_d[None, :])
            dma_weights(wv_sb, wv_d)
            for s0, w in kblocks:
                drain(proj_block_units("v", v_d, wv_sb, bv_sb, s0, w, True))
            nc.sync.dma_start(out=bq_sb, in_=bq_d[None, :])
            dma_weights(wq_sb, wq_d)
            drain(proj_block_units("q", q_d, wq_sb, bq_sb, 0, 512, True))
            # remaining consts for the spine
            nc.sync.dma_start(out=mb_sb, in_=mb_d)
            nc.sync.dma_start(out=onesr_sb, in_=onesr_d)
            nc.sync.dma_start(
                out=vh_sb[:, :, :, DK],
                in_=ki_d.rearrange("p (a b) -> p a b", a=NSK),
            )
            for j in range(NPAIR):
                nc.sync.dma_start(
                    out=wo_sb[:, j, :], in_=wo_d[j * 128 : j * 128 + 128, :]
                )

            # ---------- attention spine with woven fillers ----------
            fillers = deque()

            def weave(debt):
                while fillers and debt >= fillers[0][0]:
                    cost, u = fillers.popleft()
                    u()
                    debt -= cost
                return debt

            def emit_scores_exp(qc, pair, skc):
                sc = psum.tile([128, 1024], f32, tag="sc")
                for hh in range(2):
                    nc.tensor.matmul(
                        sc[:, hh * 512 : hh * 512 + 512],
                        lhsT=khT_sb[
                            hh * 64 : hh * 64 + 64,
                            pair,
                            skc * 128 : skc * 128 + 128,
                        ],
                        rhs=qhT_sb[
                            hh * 64 : hh * 64 + 64,
                            pair,
                            qc * 512 : qc * 512 + 512,
                        ],
                        start=True,
                        stop=True,
                    )
                probs = probpool.tile([128, 1024], BF, tag="probs")
                nc.scalar.activation(
                    out=probs,
                    in_=sc,
                    func=Exp,
                    bias=mb_sb[:, skc : skc + 1],
                    scale=0.125,
                )
                return probs

            def emit_attn(pair, skc, probs, ats):
                for hh in range(2):
                    nc.tensor.matmul(
                        ats[hh][0:65, :],
                        lhsT=vh_sb[:, skc, pair * 2 + hh, :],
                        rhs=probs[:, hh * 512 : hh * 512 + 512],
                        start=(skc == 0),
                        stop=(skc == NSK - 1),
                    )

            def make_norm(qc, pair, ats):
                def run():
                    # normalize: replicate denom, approx-reciprocal, multiply
                    rep = psum.tile([128, 1024], f32, tag="sc")
                    for hh in range(2):
                        dn = smallpool.tile([1, 512], R, tag="dn")
                        nc.vector.tensor_copy(out=dn, in_=ats[hh][64:65, :])
                        nc.tensor.matmul(
                            rep[0:64, hh * 512 : hh * 512 + 512],
                            lhsT=onesr_sb,
                            rhs=dn,
                            start=True,
                            stop=True,
                        )
                        rc = smallpool.tile([64, 512], f32, tag="rc")
                        nc.vector.reciprocal_approx_fast(
                            out=rc, in_=rep[0:64, hh * 512 : hh * 512 + 512]
                        )
                        nc.vector.tensor_mul(
                            concatT_sb[
                                hh * 64 : hh * 64 + 64,
                                pair,
                                qc * 512 : qc * 512 + 512,
                            ],
                            ats[hh][0:64, :],
                            rc,
                        )

                return run

            debt = 0.0
            pending_norm = None
            for qc in range(NQB):
                if qc + 1 < NQB:
                    fillers.extend(
                        proj_block_units(
                            "q", q_d, wq_sb, bq_sb, (qc + 1) * 512, 512, False
                        )
                    )
                for pair in range(NPAIR):
                    # peel skc=0: its scores/exp run before the previous
                    # pair's normalize so the scalar engine never idles
                    probs0 = emit_scores_exp(qc, pair, 0)
                    if pending_norm is not None:
                        pending_norm()
                        pending_norm = None
                    at0 = psum.tile([128, 512], f32, tag="at")
                    at1 = psum.tile([128, 512], f32, tag="at")
                    ats = (at0, at1)
                    emit_attn(pair, 0, probs0, ats)
                    debt = weave(debt + 0.45)
                    for skc in range(1, NSK):
                        probs = emit_scores_exp(qc, pair, skc)
                        emit_attn(pair, skc, probs, ats)
                        debt = weave(debt + 0.45)
                    pending_norm = make_norm(qc, pair, ats)
                fillers.extend(phasec_units(qc))
            pending_norm()
            # drain remaining fillers (last out-projection block)
            debt = weave(1e9)

    nc.compile()
    return nc


def get_nc(SK=SK_FAST):
    if SK not in _cache:
        _cache[SK] = _build_nc(SK)
    return _cache[SK]


def make_in_maps(q, k, v, mask, Wq, bq, Wk, bk, Wv, bv, Wo, bo):
    import ml_dtypes

    bf16 = ml_dtypes.bfloat16
    f32 = np.float32
    c = np.ascontiguousarray

    counts = [int(np.asarray(mask[b, 0]).sum()) for b in range(B)]
    SK = SK_FAST if max(counts) <= SK_FAST else S
    NSK = SK // 128

    grid = np.arange(128)[:, None] + 128 * np.arange(NSK)[None, :]  # [128,NSK]
    per_batch = []
    for b in range(B):
        idx = np.flatnonzero(np.asarray(mask[b, 0]))
        nk = len(idx)
        kc = np.zeros((SK, D), bf16)
        kc[:nk] = np.asarray(k[b], f32)[idx].astype(bf16)
        vc = np.zeros((SK, D), bf16)
        vc[:nk] = np.asarray(v[b], f32)[idx].astype(bf16)
        mb = np.where(grid < nk, 0.0, -1e9).astype(f32)
        ki = np.broadcast_to(
            (grid < nk).astype(bf16)[:, :, None], (128, NSK, 8)
        ).reshape(128, NSK * 8)
        per_batch.append(
            {
                "q": np.asarray(q[b], f32).astype(bf16),
                "kc": kc,
                "vc": vc,
                "maskbias": mb,
                "keyind": c(ki),
            }
        )

    ones = np.ones((128, 512), bf16)
    onesr = np.ones((1, 64), f32)
    ident = np.eye(128, dtype=bf16)
    in_maps = []
    for core in range(NCORES):
        b, g = core // 2, core % 2
        cols = slice(g * CPG, (g + 1) * CPG)
        m = dict(per_batch[b])
        m.update(
            {
                "wq": np.asarray(Wq[:, cols], f32).astype(bf16),
                "wk": np.asarray(Wk[:, cols], f32).astype(bf16),
                "wv": np.asarray(Wv[:, cols], f32).astype(bf16),
                "wo": np.asarray(Wo[cols, :], f32).astype(bf16),
                "bq": np.asarray(bq[cols], f32).astype(bf16),
                "bk": np.asarray(bk[cols], f32).astype(bf16),
                "bv": np.asarray(bv[cols], f32).astype(bf16),
                "ones": ones,
                "onesr": onesr,
                "ident": ident,
            }
        )
        in_maps.append(m)
    return in_maps, SK


def gather(results, bo):
    out = np.zeros((B, S, D), np.float32)
    for core in range(NCORES):
        b = core // 2
        out[b] += results[core]["out"]
    out += np.asarray(bo, np.float32)[None, None, :]
    return out


def run_on_hw(in_maps, SK=SK_FAST, trace=False, trace_cores=None):
    from concourse.bass_utils import run_bass_kernel_spmd

    nc = get_nc(SK)
    return run_bass_kernel_spmd(
        nc,
        in_maps,
        list(range(NCORES)),
        trace=trace,
        trace_cores=trace_cores,
    )


def kernel(q, k, v, mask, Wq, bq, Wk, bk, Wv, bv, Wo, bo):
    in_maps, SK = make_in_maps(q, k, v, mask, Wq, bq, Wk, bk, Wv, bv, Wo, bo)
    res = run_on_hw(in_maps, SK)
    return gather(res.results, bo)


# revision 16
# speedup vs baseline: 1.2451x; 1.2451x over previous
"""Multi-head attention (B=4, S=2048, D=1024, H=16, Dk=64) on 8 trn2 NeuronCores.

Sharding: core = (batch b, head-group g), g selects 8 heads (512 proj cols).
Host sums the two partial out-projections per batch and adds bo.

Key optimizations over the v1 kernel (912us):
  * Host-side key compaction: masked keys give exactly-zero probs in the
    reference (exp(-1e9/8) underflows), so drop them on the host and pad
    k/v to SK=1152 rows (mask is Bernoulli(0.5), so ~1024 survive; fall
    back to SK=2048 if a batch ever exceeds 1152).  Cuts k/v projections,
    scores, attn and the scalar-engine exp work by ~44%.
  * bf16 matmuls everywhere (host pre-casts inputs/weights): same 1
    col/cycle stream rate as f32r but fast weight loads (FWL), half the
    DMA and SBUF footprint.  fp32 accumulation in PSUM.
  * Row-tiled scores: the K=64 scores matmuls of the two heads of a pair
    run concurrently in PE row groups 0/64 (tile_position auto-derived
    from the partition bases) -> 2x PE throughput on scores.
  * Pad-key handling via a per-chunk -1e9 activation bias (probs of pad
    keys are exactly 0), plus an indicator column in vh giving the
    softmax denominator for free (row 64 of the attn accumulator).
  * Normalization via replicate-denominator matmul + reciprocal_approx_fast
    on 64 partitions (the v1 kernel burned 6.5us per [1,1024] serial
    reciprocal).
  * Software-pipelined emission: the q projections for query blocks 1-3
    and the out-projection (phase C) are woven into the ACT-bound
    attention loop as PE filler so the tensor engine never idles (keeps
    the HAM clock gate at 2.4 GHz; the v1 kernel sat at 1.2 GHz for
    600us of its runtime).
"""

import sys

sys.path.insert(0, "/opt/trn_rl_repo")

import numpy as np

B, S, D, H, DK = 4, 2048, 1024, 16, 64
CPG = 512          # projection columns per core (8 heads x 64)
NCORES = 8
SK_FAST = 1152     # compacted+padded key rows (multiple of 128)

_cache = {}


def _build_nc(SK):
    import contextlib
    from collections import deque

    import concourse.bass as bass
    import concourse.tile as tile
    from concourse import bacc, mybir

    f32 = mybir.dt.float32
    R = mybir.dt.float32r
    BF = mybir.dt.bfloat16
    Exp = mybir.ActivationFunctionType.Exp

    NSK = SK // 128        # key chunks of 128
    NQB = S // 512         # query 512-blocks (4)
    NDCH = D // 128        # contraction chunks for projections (8)
    NPAIR = 4              # head pairs per core

    nc = bacc.Bacc("TRN2", target_bir_lowering=False, debug=False)

    q_d = nc.dram_tensor("q", [S, D], BF, kind="ExternalInput").ap()
    k_d = nc.dram_tensor("kc", [SK, D], BF, kind="ExternalInput").ap()
    v_d = nc.dram_tensor("vc", [SK, D], BF, kind="ExternalInput").ap()
    wq_d = nc.dram_tensor("wq", [D, CPG], BF, kind="ExternalInput").ap()
    wk_d = nc.dram_tensor("wk", [D, CPG], BF, kind="ExternalInput").ap()
    wv_d = nc.dram_tensor("wv", [D, CPG], BF, kind="ExternalInput").ap()
    wo_d = nc.dram_tensor("wo", [CPG, D], BF, kind="ExternalInput").ap()
    bq_d = nc.dram_tensor("bq", [CPG], BF, kind="ExternalInput").ap()
    bk_d = nc.dram_tensor("bk", [CPG], BF, kind="ExternalInput").ap()
    bv_d = nc.dram_tensor("bv", [CPG], BF, kind="ExternalInput").ap()
    mb_d = nc.dram_tensor("maskbias", [128, NSK], f32, kind="ExternalInput").ap()
    ki_d = nc.dram_tensor("keyind", [128, NSK * 8], BF, kind="ExternalInput").ap()
    ones_d = nc.dram_tensor("ones", [128, 512], BF, kind="ExternalInput").ap()
    onesr_d = nc.dram_tensor("onesr", [1, 64], R, kind="ExternalInput").ap()
    ident_d = nc.dram_tensor("ident", [128, 128], BF, kind="ExternalInput").ap()
    out_d = nc.dram_tensor("out", [S, D], f32, kind="ExternalOutput").ap()

    with tile.TileContext(nc) as tc:
        import contextlib

        with contextlib.ExitStack() as ctx:
            # ---------- persistent tensors + constants ----------
            persist = ctx.enter_context(tc.tile_pool(name="persist", bufs=1))
            consts = ctx.enter_context(tc.tile_pool(name="consts", bufs=1))

            qhT_sb = persist.tile([128, NPAIR, S], BF)     # [c%128, pair, sq]
            khT_sb = persist.tile([128, NPAIR, SK], BF)
            vh_sb = persist.tile([128, NSK, 8, DK + 1], BF)  # ind col at 64
            concatT_sb = persist.tile([128, NPAIR, S], BF)
            wq_sb = persist.tile([128, NDCH, CPG], BF)
            wk_sb = persist.tile([128, NDCH, CPG], BF)
            wv_sb = persist.tile([128, NDCH, CPG], BF)
            wo_sb = persist.tile([128, NPAIR, D], BF)

            ones_sb = consts.tile([1, 512], BF)
            onesr_sb = consts.tile([1, 64], R)
            ident = consts.tile([128, 128], BF)
            mb_sb = consts.tile([128, NSK], f32)
            bq_sb = consts.tile([1, CPG], BF)
            bk_sb = consts.tile([1, CPG], BF)
            bv_sb = consts.tile([1, CPG], BF)

            # DMA emission is ordered so the PE can start within a few us:
            # first k x-tiles + what the first transposes/projections need;
            # the rest of the weights just-in-time before their consumers.
            def dma_weights(t_sb, t_d):
                for j in range(NDCH):
                    nc.sync.dma_start(
                        out=t_sb[:, j, :], in_=t_d[j * 128 : j * 128 + 128, :]
                    )

            # ---------- shared SBUF rings ----------
            natpool = ctx.enter_context(tc.tile_pool(name="natpool", bufs=8))
            xtpool = ctx.enter_context(tc.tile_pool(name="xtpool", bufs=10))
            probpool = ctx.enter_context(tc.tile_pool(name="probpool", bufs=3))
            smallpool = ctx.enter_context(tc.tile_pool(name="smallpool", bufs=4))
            outpool = ctx.enter_context(tc.tile_pool(name="outpool", bufs=3))

            # PSUM pools are phase-local: the prefix gets deep tp/pr rings
            # (6 banks); the spine re-uses those banks as
            # sc 2x[128,1024]f32 (4) + at 2x[128,512]f32 (2) +
            # fill 2x[128,512] (2) = 8 banks exactly.
            pools = {}

            # ---------- projection block emitters ----------
            def proj_block_units(kind, x_d, w_sb, b_sb, s0, w, act_copy,
                                 nats_in=None):
                """Generate unit-closures for projecting x rows [s0, s0+w).
                kind: 'q'/'k' -> [c, s] into qhT_sb/khT_sb; 'v' -> vh_sb."""
                nsub = w // 128
                nats = nats_in if nats_in is not None else []
                xts = []

                def u_load():
                    for i in range(nsub):
                        nat = natpool.tile([128, D], BF, tag="nat")
                        r0 = s0 + i * 128
                        nc.sync.dma_start(out=nat, in_=x_d[r0 : r0 + 128, :])
                        nats.append(nat)

                if nats_in is None:
                    yield 0.1, u_load

                def u_tp(j):
                    def run():
                        tp = pools["tp"]()
                        for i in range(nsub):
                            nc.tensor.transpose(
                                out=tp[:, i * 128 : i * 128 + 128],
                                in_=nats[i][:, j * 128 : j * 128 + 128],
                                identity=ident,
                            )
                        xt = xtpool.tile([128, 512], BF, tag="xt")
                        if act_copy:
                            nc.scalar.copy(out=xt[:, :w], in_=tp[:, :w])
                        else:
                            nc.vector.tensor_copy(out=xt[:, :w], in_=tp[:, :w])
                        xts.append(xt)

                    return run

                for j in range(NDCH):
                    yield 0.3, u_tp(j)

                if kind in ("q", "k"):
                    dst = qhT_sb if kind == "q" else khT_sb

                    def u_proj_a(cch, box):
                        def run():
                            pr = pools["pr"]()
                            box.append(pr)
                            nc.tensor.matmul(
                                pr[:, :w],
                                lhsT=b_sb[0:1, cch * 128 : cch * 128 + 128],
                                rhs=ones_sb[0:1, :w],
                                start=True,
                                stop=False,
                            )
                            for j in range(NDCH // 2):
                                nc.tensor.matmul(
                                    pr[:, :w],
                                    lhsT=w_sb[:, j, cch * 128 : cch * 128 + 128],
                                    rhs=xts[j][:, :w],
                                    start=False,
                                    stop=False,
                                )

                        return run

                    def u_proj_b(cch, box):
                        def run():
                            pr = box.pop()
                            for j in range(NDCH // 2, NDCH):
                                nc.tensor.matmul(
                                    pr[:, :w],
                                    lhsT=w_sb[:, j, cch * 128 : cch * 128 + 128],
                                    rhs=xts[j][:, :w],
                                    start=False,
                                    stop=(j == NDCH - 1),
                                )
                            nc.vector.tensor_copy(
                                out=dst[:, cch, s0 : s0 + w], in_=pr[:, :w]
                            )

                        return run

                    for cch in range(NPAIR):
                        box = []
                        yield 1.0, u_proj_a(cch, box)
                        yield 1.0, u_proj_b(cch, box)
                else:

                    def u_projv_a(sub, box):
                        def run():
                            pr = pools["pr"]()
                            box.append(pr)
                            nc.tensor.matmul(
                                pr,
                                lhsT=ones_sb[0:1, 0:128],
                                rhs=b_sb[0:1, :],
                                start=True,
                                stop=False,
                            )
                            for j in range(NDCH // 2):
                                nc.tensor.matmul(
                                    pr,
                                    lhsT=xts[j][:, sub * 128 : sub * 128 + 128],
                                    rhs=w_sb[:, j, :],
                                    start=False,
                                    stop=False,
                                )

                        return run

                    def u_projv_b(sub, box):
                        def run():
                            pr = box.pop()
                            for j in range(NDCH // 2, NDCH):
                                nc.tensor.matmul(
                                    pr,
                                    lhsT=xts[j][:, sub * 128 : sub * 128 + 128],
                                    rhs=w_sb[:, j, :],
                                    start=False,
                                    stop=(j == NDCH - 1),
                                )
                            skc = (s0 + sub * 128) // 128
                            nc.vector.tensor_copy(
                                out=vh_sb[:, skc, :, 0:DK],
                                in_=pr.rearrange("p (h d) -> p h d", h=8),
                            )

                        return run

                    for sub in range(nsub):
                        box = []
                        yield 1.0, u_projv_a(sub, box)
                        yield 1.0, u_projv_b(sub, box)

            def phasec_units(qc):
                """Out-projection for query block qc (concatT -> out)."""

                def u_cblk(sqc, do):
                    def run():
                        o_ps = spsum.tile([128, 512], f32, tag="fill")
                        for p in range(NPAIR):
                            nc.tensor.matmul(
                                o_ps,
                                lhsT=concatT_sb[
                                    :, p, sqc * 128 : sqc * 128 + 128
                                ],
                                rhs=wo_sb[:, p, do * 512 : do * 512 + 512],
                                start=(p == 0),
                                stop=(p == NPAIR - 1),
                            )
                        o_sb = outpool.tile([128, 512], f32, tag="osb")
                        nc.vector.tensor_copy(out=o_sb, in_=o_ps)
                        nc.sync.dma_start(
                            out=out_d[
                                sqc * 128 : sqc * 128 + 128,
                                do * 512 : do * 512 + 512,
                            ],
                            in_=o_sb,
                        )

                    return run

                for sq in range(4):
                    for do in range(2):
                        yield 1.0, u_cblk(qc * 4 + sq, do)

            def drain(units):
                for _, u in units:
                    u()

            # ---------- prefix: k, v and q block 0 (ACT does the copies) ----
            prefix_ctx = contextlib.ExitStack()
            ppsum = prefix_ctx.enter_context(
                tc.tile_pool(name="ppsum", bufs=2, space="PSUM")
            )
            pools["tp"] = lambda: ppsum.tile([128, 512], BF, tag="tp", bufs=2, name="tp")
            pools["pr"] = lambda: ppsum.tile([128, 512], f32, tag="pr", bufs=4, name="pr")
            kblocks = []
            o = 0
            while o < SK:
                w = min(512, SK - o)
                kblocks.append((o, w))
                o += w

            # first k x-tiles + immediate deps, then weights just-in-time
            knats0 = []
            for i in range(4):
                nat = natpool.tile([128, D], BF, tag="nat")
                nc.sync.dma_start(out=nat, in_=k_d[i * 128 : i * 128 + 128, :])
                knats0.append(nat)
            nc.sync.dma_start(out=ident, in_=ident_d)
            nc.sync.dma_start(out=ones_sb, in_=ones_d[0:1, :])
            nc.sync.dma_start(out=bk_sb, in_=bk_d[None, :])
            dma_weights(wk_sb, wk_d)
            nc.sync.dma_start(out=mb_sb, in_=mb_d)
            nc.sync.dma_start(out=onesr_sb, in_=onesr_d)
            nc.sync.dma_start(
                out=vh_sb[:, :, :, DK],
                in_=ki_d.rearrange("p (a b) -> p a b", a=NSK),
            )
            for j in range(NPAIR):
                nc.sync.dma_start(
                    out=wo_sb[:, j, :], in_=wo_d[j * 128 : j * 128 + 128, :]
                )
            for s0, w in kblocks:
                drain(
                    proj_block_units(
                        "k", k_d, wk_sb, bk_sb, s0, w, True,
                        nats_in=knats0 if s0 == 0 else None,
                    )
                )
            nc.sync.dma_start(out=bv_sb, in_=bv_d[None, :])
            dma_weights(wv_sb, wv_d)
            for s0, w in kblocks:
                drain(proj_block_units("v", v_d, wv_sb, bv_sb, s0, w, True))
            nc.sync.dma_start(out=bq_sb, in_=bq_d[None, :])
            dma_weights(wq_sb, wq_d)
            drain(proj_block_units("q", q_d, wq_sb, bq_sb, 0, 512, True))

            # ---------- attention spine with woven fillers ----------
            prefix_ctx.close()
            spsum = ctx.enter_context(
                tc.tile_pool(name="spsum", bufs=2, space="PSUM")
            )
            pools["tp"] = lambda: spsum.tile([128, 512], BF, tag="fill", name="tpw")
            pools["pr"] = lambda: spsum.tile([128, 512], f32, tag="fill", name="prw")
            fillers = deque()

            def weave(debt):
                while fillers and debt >= fillers[0][0]:
                    cost, u = fillers.popleft()
                    u()
                    debt -= cost
                return debt

            def emit_scores_exp(qc, pair, skc):
                sc = spsum.tile([128, 1024], f32, tag="sc")
                for hh in range(2):
                    nc.tensor.matmul(
                        sc[:, hh * 512 : hh * 512 + 512],
                        lhsT=khT_sb[
                            hh * 64 : hh * 64 + 64,
                            pair,
                            skc * 128 : skc * 128 + 128,
                        ],
                        rhs=qhT_sb[
                            hh * 64 : hh * 64 + 64,
                            pair,
                            qc * 512 : qc * 512 + 512,
                        ],
                        start=True,
                        stop=True,
                    )
                probs = probpool.tile([128, 1024], BF, tag="probs")
                nc.scalar.activation(
                    out=probs,
                    in_=sc,
                    func=Exp,
                    bias=mb_sb[:, skc : skc + 1],
                    scale=0.125,
                )
                return probs

            def emit_attn(pair, skc, probs, ats):
                for hh in range(2):
                    nc.tensor.matmul(
                        ats[hh][0:65, :],
                        lhsT=vh_sb[:, skc, pair * 2 + hh, :],
                        rhs=probs[:, hh * 512 : hh * 512 + 512],
                        start=(skc == 0),
                        stop=(skc == NSK - 1),
                    )

            def make_norm(qc, pair, ats):
                def run():
                    # normalize: replicate denom, approx-reciprocal, multiply
                    rep = spsum.tile([128, 1024], f32, tag="sc")
                    for hh in range(2):
                        dn = smallpool.tile([1, 512], R, tag="dn")
                        nc.vector.tensor_copy(out=dn, in_=ats[hh][64:65, :])
                        nc.tensor.matmul(
                            rep[0:64, hh * 512 : hh * 512 + 512],
                            lhsT=onesr_sb,
                            rhs=dn,
                            start=True,
                            stop=True,
                        )
                        rc = smallpool.tile([64, 512], f32, tag="rc")
                        nc.vector.reciprocal_approx_fast(
                            out=rc, in_=rep[0:64, hh * 512 : hh * 512 + 512]
                        )
                        nc.vector.tensor_mul(
                            concatT_sb[
                                hh * 64 : hh * 64 + 64,
                                pair,
                                qc * 512 : qc * 512 + 512,
                            ],
                            ats[hh][0:64, :],
                            rc,
                        )

                return run

            debt = 0.0
            pending_norm = None
            for qc in range(NQB):
                if qc + 1 < NQB:
                    fillers.extend(
                        proj_block_units(
                            "q", q_d, wq_sb, bq_sb, (qc + 1) * 512, 512, False
                        )
                    )
                for pair in range(NPAIR):
                    # scores run one sk-step ahead of attn, and the previous
                    # pair's normalize is deferred past this pair's first two
                    # score/exp steps, so the scalar engine never idles and
                    # woven fillers never delay the next exp's input.
                    probs_prev = emit_scores_exp(qc, pair, 0)
                    probs_cur = emit_scores_exp(qc, pair, 1)
                    if pending_norm is not None:
                        pending_norm()
                        pending_norm = None
                    at0 = spsum.tile([128, 512], f32, tag="at")
                    at1 = spsum.tile([128, 512], f32, tag="at")
                    ats = (at0, at1)
                    emit_attn(pair, 0, probs_prev, ats)
                    debt = weave(debt + 0.9)
                    for skc in range(2, NSK):
                        probs_next = emit_scores_exp(qc, pair, skc)
                        emit_attn(pair, skc - 1, probs_cur, ats)
                        probs_cur = probs_next
                        debt = weave(debt + 0.45)
                    emit_attn(pair, NSK - 1, probs_cur, ats)
                    debt = weave(debt + 0.45)
                    pending_norm = make_norm(qc, pair, ats)
                fillers.extend(phasec_units(qc))
            pending_norm()
            # drain remaining fillers (last out-projection block)
            debt = weave(1e9)

    nc.compile()
    return nc


def get_nc(SK=SK_FAST):
    if SK not in _cache:
        _cache[SK] = _build_nc(SK)
    return _cache[SK]


def make_in_maps(q, k, v, mask, Wq, bq, Wk, bk, Wv, bv, Wo, bo):
    import ml_dtypes

    bf16 = ml_dtypes.bfloat16
    f32 = np.float32
    c = np.ascontiguousarray

    counts = [int(np.asarray(mask[b, 0]).sum()) for b in range(B)]
    SK = SK_FAST if max(counts) <= SK_FAST else S
    NSK = SK // 128

    grid = np.arange(128)[:, None] + 128 * np.arange(NSK)[None, :]  # [128,NSK]
    per_batch = []
    for b in range(B):
        idx = np.flatnonzero(np.asarray(mask[b, 0]))
        nk = len(idx)
        kc = np.zeros((SK, D), bf16)
        kc[:nk] = np.asarray(k[b], f32)[idx].astype(bf16)
        vc = np.zeros((SK, D), bf16)
        vc[:nk] = np.asarray(v[b], f32)[idx].astype(bf16)
        mb = np.where(grid < nk, 0.0, -1e9).astype(f32)
        ki = np.broadcast_to(
            (grid < nk).astype(bf16)[:, :, None], (128, NSK, 8)
        ).reshape(128, NSK * 8)
        per_batch.append(
            {
                "q": np.asarray(q[b], f32).astype(bf16),
                "kc": kc,
                "vc": vc,
                "maskbias": mb,
                "keyind": c(ki),
            }
        )

    ones = np.ones((128, 512), bf16)
    onesr = np.ones((1, 64), f32)
    ident = np.eye(128, dtype=bf16)
    in_maps = []
    for core in range(NCORES):
        b, g = core // 2, core % 2
        cols = slice(g * CPG, (g + 1) * CPG)
        m = dict(per_batch[b])
        m.update(
            {
                "wq": np.asarray(Wq[:, cols], f32).astype(bf16),
                "wk": np.asarray(Wk[:, cols], f32).astype(bf16),
                "wv": np.asarray(Wv[:, cols], f32).astype(bf16),
                "wo": np.asarray(Wo[cols, :], f32).astype(bf16),
                "bq": np.asarray(bq[cols], f32).astype(bf16),
                "bk": np.asarray(bk[cols], f32).astype(bf16),
                "bv": np.asarray(bv[cols], f32).astype(bf16),
                "ones": ones,
                "onesr": onesr,
                "ident": ident,
            }
        )
        in_maps.append(m)
    return in_maps, SK


def gather(results, bo):
    out = np.zeros((B, S, D), np.float32)
    for core in range(NCORES):
        b = core // 2
        out[b] += results[core]["out"]
    out += np.asarray(bo, np.float32)[None, None, :]
    return out


def run_on_hw(in_maps, SK=SK_FAST, trace=False, trace_cores=None):
    from concourse.bass_utils import run_bass_kernel_spmd

    nc = get_nc(SK)
    return run_bass_kernel_spmd(
        nc,
        in_maps,
        list(range(NCORES)),
        trace=trace,
        trace_cores=trace_cores,
    )


def kernel(q, k, v, mask, Wq, bq, Wk, bk, Wv, bv, Wo, bo):
    in_maps, SK = make_in_maps(q, k, v, mask, Wq, bq, Wk, bk, Wv, bv, Wo, bo)
    res = run_on_hw(in_maps, SK)
    return gather(res.results, bo)


# revision 17
# speedup vs baseline: 1.2504x; 1.0042x over previous
"""Multi-head attention (B=4, S=2048, D=1024, H=16, Dk=64) on 8 trn2 NeuronCores.

Sharding: core = (batch b, head-group g), g selects 8 heads (512 proj cols).
Host sums the two partial out-projections per batch and adds bo.

Key optimizations over the v1 kernel (912us):
  * Host-side key compaction: masked keys give exactly-zero probs in the
    reference (exp(-1e9/8) underflows), so drop them on the host and pad
    k/v to SK=1152 rows (mask is Bernoulli(0.5), so ~1024 survive; fall
    back to SK=2048 if a batch ever exceeds 1152).  Cuts k/v projections,
    scores, attn and the scalar-engine exp work by ~44%.
  * bf16 matmuls everywhere (host pre-casts inputs/weights): same 1
    col/cycle stream rate as f32r but fast weight loads (FWL), half the
    DMA and SBUF footprint.  fp32 accumulation in PSUM.
  * Row-tiled scores: the K=64 scores matmuls of the two heads of a pair
    run concurrently in PE row groups 0/64 (tile_position auto-derived
    from the partition bases) -> 2x PE throughput on scores.
  * Pad-key handling via a per-chunk -1e9 activation bias (probs of pad
    keys are exactly 0), plus an indicator column in vh giving the
    softmax denominator for free (row 64 of the attn accumulator).
  * Normalization via replicate-denominator matmul + reciprocal_approx_fast
    on 64 partitions (the v1 kernel burned 6.5us per [1,1024] serial
    reciprocal).
  * Software-pipelined emission: the q projections for query blocks 1-3
    and the out-projection (phase C) are woven into the ACT-bound
    attention loop as PE filler so the tensor engine never idles (keeps
    the HAM clock gate at 2.4 GHz; the v1 kernel sat at 1.2 GHz for
    600us of its runtime).
"""

import sys

sys.path.insert(0, "/opt/trn_rl_repo")

import numpy as np

B, S, D, H, DK = 4, 2048, 1024, 16, 64
CPG = 512          # projection columns per core (8 heads x 64)
NCORES = 8
SK_FAST = 1152     # compacted+padded key rows (multiple of 128)

_cache = {}


def _build_nc(SK):
    import contextlib
    from collections import deque

    import concourse.bass as bass
    import concourse.tile as tile
    from concourse import bacc, mybir

    f32 = mybir.dt.float32
    R = mybir.dt.float32r
    BF = mybir.dt.bfloat16
    Exp = mybir.ActivationFunctionType.Exp

    NSK = SK // 128        # key chunks of 128
    NQB = S // 512         # query 512-blocks (4)
    NDCH = D // 128        # contraction chunks for projections (8)
    NPAIR = 4              # head pairs per core

    nc = bacc.Bacc("TRN2", target_bir_lowering=False, debug=False)

    q_d = nc.dram_tensor("q", [S, D], BF, kind="ExternalInput").ap()
    k_d = nc.dram_tensor("kc", [SK, D], BF, kind="ExternalInput").ap()
    v_d = nc.dram_tensor("vc", [SK, D], BF, kind="ExternalInput").ap()
    wq_d = nc.dram_tensor("wq", [D, CPG], BF, kind="ExternalInput").ap()
    wk_d = nc.dram_tensor("wk", [D, CPG], BF, kind="ExternalInput").ap()
    wv_d = nc.dram_tensor("wv", [D, CPG], BF, kind="ExternalInput").ap()
    wo_d = nc.dram_tensor("wo", [CPG, D], BF, kind="ExternalInput").ap()
    bq_d = nc.dram_tensor("bq", [CPG], BF, kind="ExternalInput").ap()
    bk_d = nc.dram_tensor("bk", [CPG], BF, kind="ExternalInput").ap()
    bv_d = nc.dram_tensor("bv", [CPG], BF, kind="ExternalInput").ap()
    mb_d = nc.dram_tensor("maskbias", [128, NSK], f32, kind="ExternalInput").ap()
    ki_d = nc.dram_tensor("keyind", [128, NSK * 8], BF, kind="ExternalInput").ap()
    ones_d = nc.dram_tensor("ones", [128, 512], BF, kind="ExternalInput").ap()
    onesr_d = nc.dram_tensor("onesr", [1, 64], R, kind="ExternalInput").ap()
    ident_d = nc.dram_tensor("ident", [128, 128], BF, kind="ExternalInput").ap()
    out_d = nc.dram_tensor("out", [S, D], f32, kind="ExternalOutput").ap()

    with tile.TileContext(nc) as tc:
        import contextlib

        with contextlib.ExitStack() as ctx:
            # ---------- persistent tensors + constants ----------
            persist = ctx.enter_context(tc.tile_pool(name="persist", bufs=1))
            consts = ctx.enter_context(tc.tile_pool(name="consts", bufs=1))

            qhT_sb = persist.tile([128, NPAIR, S], BF)     # [c%128, pair, sq]
            khT_sb = persist.tile([128, NPAIR, SK], BF)
            vh_sb = persist.tile([128, NSK, 8, DK + 1], BF)  # ind col at 64
            concatT_sb = persist.tile([128, NPAIR, S], BF)
            wq_sb = persist.tile([128, NDCH, CPG], BF)
            wk_sb = persist.tile([128, NDCH, CPG], BF)
            wv_sb = persist.tile([128, NDCH, CPG], BF)
            wo_sb = persist.tile([128, NPAIR, D], BF)

            ones_sb = consts.tile([1, 512], BF)
            onesr_sb = consts.tile([1, 64], R)
            ident = consts.tile([128, 128], BF)
            mb_sb = consts.tile([128, NSK], f32)
            bq_sb = consts.tile([1, CPG], BF)
            bk_sb = consts.tile([1, CPG], BF)
            bv_sb = consts.tile([1, CPG], BF)

            # DMA emission is ordered so the PE can start within a few us:
            # first k x-tiles + what the first transposes/projections need;
            # the rest of the weights just-in-time before their consumers.
            def dma_weights(t_sb, t_d):
                for j in range(NDCH):
                    nc.sync.dma_start(
                        out=t_sb[:, j, :], in_=t_d[j * 128 : j * 128 + 128, :]
                    )

            # ---------- shared SBUF rings ----------
            natpool = ctx.enter_context(tc.tile_pool(name="natpool", bufs=8))
            xtpool = ctx.enter_context(tc.tile_pool(name="xtpool", bufs=10))
            probpool = ctx.enter_context(tc.tile_pool(name="probpool", bufs=3))
            smallpool = ctx.enter_context(tc.tile_pool(name="smallpool", bufs=4))
            outpool = ctx.enter_context(tc.tile_pool(name="outpool", bufs=3))

            # PSUM pools are phase-local: the prefix gets deep tp/pr rings
            # (6 banks); the spine re-uses those banks as
            # sc 2x[128,1024]f32 (4) + at 2x[128,512]f32 (2) +
            # fill 2x[128,512] (2) = 8 banks exactly.
            pools = {}

            # ---------- projection block emitters ----------
            def proj_block_units(kind, x_d, w_sb, b_sb, s0, w, act_copy,
                                 nats_in=None):
                """Generate unit-closures for projecting x rows [s0, s0+w).
                kind: 'q'/'k' -> [c, s] into qhT_sb/khT_sb; 'v' -> vh_sb."""
                nsub = w // 128
                nats = nats_in if nats_in is not None else []
                xts = []

                def u_load():
                    for i in range(nsub):
                        nat = natpool.tile([128, D], BF, tag="nat")
                        r0 = s0 + i * 128
                        nc.sync.dma_start(out=nat, in_=x_d[r0 : r0 + 128, :])
                        nats.append(nat)

                if nats_in is None:
                    yield 0.1, u_load

                def u_tp(j):
                    def run():
                        tp = pools["tp"]()
                        for i in range(nsub):
                            nc.tensor.transpose(
                                out=tp[:, i * 128 : i * 128 + 128],
                                in_=nats[i][:, j * 128 : j * 128 + 128],
                                identity=ident,
                            )
                        xt = xtpool.tile([128, 512], BF, tag="xt")
                        if act_copy:
                            nc.scalar.copy(out=xt[:, :w], in_=tp[:, :w])
                        else:
                            nc.vector.tensor_copy(out=xt[:, :w], in_=tp[:, :w])
                        xts.append(xt)

                    return run

                for j in range(NDCH):
                    yield 0.3, u_tp(j)

                if kind in ("q", "k"):
                    dst = qhT_sb if kind == "q" else khT_sb

                    def u_proj(cch, box, j0, j1):
                        def run():
                            if j0 == 0:
                                pr = pools["pr"]()
                                box.append(pr)
                                nc.tensor.matmul(
                                    pr[:, :w],
                                    lhsT=b_sb[0:1, cch * 128 : cch * 128 + 128],
                                    rhs=ones_sb[0:1, :w],
                                    start=True,
                                    stop=False,
                                )
                            pr = box[0]
                            for j in range(j0, j1):
                                nc.tensor.matmul(
                                    pr[:, :w],
                                    lhsT=w_sb[:, j, cch * 128 : cch * 128 + 128],
                                    rhs=xts[j][:, :w],
                                    start=False,
                                    stop=(j == NDCH - 1),
                                )
                            if j1 == NDCH:
                                nc.vector.tensor_copy(
                                    out=dst[:, cch, s0 : s0 + w], in_=pr[:, :w]
                                )

                        return run

                    for cch in range(NPAIR):
                        box = []
                        for j0 in range(0, NDCH, 2):
                            yield 0.5, u_proj(cch, box, j0, j0 + 2)
                else:

                    def u_projv(sub, box, j0, j1):
                        def run():
                            if j0 == 0:
                                pr = pools["pr"]()
                                box.append(pr)
                                nc.tensor.matmul(
                                    pr,
                                    lhsT=ones_sb[0:1, 0:128],
                                    rhs=b_sb[0:1, :],
                                    start=True,
                                    stop=False,
                                )
                            pr = box[0]
                            for j in range(j0, j1):
                                nc.tensor.matmul(
                                    pr,
                                    lhsT=xts[j][:, sub * 128 : sub * 128 + 128],
                                    rhs=w_sb[:, j, :],
                                    start=False,
                                    stop=(j == NDCH - 1),
                                )
                            if j1 == NDCH:
                                skc = (s0 + sub * 128) // 128
                                nc.vector.tensor_copy(
                                    out=vh_sb[:, skc, :, 0:DK],
                                    in_=pr.rearrange("p (h d) -> p h d", h=8),
                                )

                        return run

                    for sub in range(nsub):
                        box = []
                        for j0 in range(0, NDCH, 2):
                            yield 0.5, u_projv(sub, box, j0, j0 + 2)

            def phasec_units(qc):
                """Out-projection for query block qc (concatT -> out)."""

                def u_cblk(sqc, do, half, box):
                    def run():
                        if half == 0:
                            box.append(spsum.tile([128, 512], f32, tag="fill",
                                                  name="o_ps"))
                        o_ps = box[0]
                        for p in (0, 1) if half == 0 else (2, 3):
                            nc.tensor.matmul(
                                o_ps,
                                lhsT=concatT_sb[
                                    :, p, sqc * 128 : sqc * 128 + 128
                                ],
                                rhs=wo_sb[:, p, do * 512 : do * 512 + 512],
                                start=(p == 0),
                                stop=(p == NPAIR - 1),
                            )
                        if half == 1:
                            o_sb = outpool.tile([128, 512], f32, tag="osb")
                            nc.vector.tensor_copy(out=o_sb, in_=o_ps)
                            nc.sync.dma_start(
                                out=out_d[
                                    sqc * 128 : sqc * 128 + 128,
                                    do * 512 : do * 512 + 512,
                                ],
                                in_=o_sb,
                            )

                    return run

                for sq in range(4):
                    for do in range(2):
                        box = []
                        for half in range(2):
                            yield 0.5, u_cblk(qc * 4 + sq, do, half, box)

            def drain(units):
                for _, u in units:
                    u()

            # ---------- prefix: k, v and q block 0 (ACT does the copies) ----
            prefix_ctx = contextlib.ExitStack()
            ppsum = prefix_ctx.enter_context(
                tc.tile_pool(name="ppsum", bufs=2, space="PSUM")
            )
            pools["tp"] = lambda: ppsum.tile([128, 512], BF, tag="tp", bufs=2, name="tp")
            pools["pr"] = lambda: ppsum.tile([128, 512], f32, tag="pr", bufs=4, name="pr")
            kblocks = []
            o = 0
            while o < SK:
                w = min(512, SK - o)
                kblocks.append((o, w))
                o += w

            # first k x-tiles + immediate deps, then weights just-in-time
            knats0 = []
            for i in range(4):
                nat = natpool.tile([128, D], BF, tag="nat")
                nc.sync.dma_start(out=nat, in_=k_d[i * 128 : i * 128 + 128, :])
                knats0.append(nat)
            nc.sync.dma_start(out=ident, in_=ident_d)
            nc.sync.dma_start(out=ones_sb, in_=ones_d[0:1, :])
            nc.sync.dma_start(out=bk_sb, in_=bk_d[None, :])
            dma_weights(wk_sb, wk_d)
            nc.sync.dma_start(out=mb_sb, in_=mb_d)
            nc.sync.dma_start(out=onesr_sb, in_=onesr_d)
            nc.sync.dma_start(
                out=vh_sb[:, :, :, DK],
                in_=ki_d.rearrange("p (a b) -> p a b", a=NSK),
            )
            for j in range(NPAIR):
                nc.sync.dma_start(
                    out=wo_sb[:, j, :], in_=wo_d[j * 128 : j * 128 + 128, :]
                )
            for s0, w in kblocks:
                drain(
                    proj_block_units(
                        "k", k_d, wk_sb, bk_sb, s0, w, True,
                        nats_in=knats0 if s0 == 0 else None,
                    )
                )
            nc.sync.dma_start(out=bv_sb, in_=bv_d[None, :])
            dma_weights(wv_sb, wv_d)
            for s0, w in kblocks:
                drain(proj_block_units("v", v_d, wv_sb, bv_sb, s0, w, True))
            nc.sync.dma_start(out=bq_sb, in_=bq_d[None, :])
            dma_weights(wq_sb, wq_d)
            drain(proj_block_units("q", q_d, wq_sb, bq_sb, 0, 512, True))

            # ---------- attention spine with woven fillers ----------
            prefix_ctx.close()
            spsum = ctx.enter_context(
                tc.tile_pool(name="spsum", bufs=2, space="PSUM")
            )
            pools["tp"] = lambda: spsum.tile([128, 512], BF, tag="fill", name="tpw")
            pools["pr"] = lambda: spsum.tile([128, 512], f32, tag="fill", name="prw")
            fillers = deque()

            def weave(debt):
                while fillers and debt >= fillers[0][0]:
                    cost, u = fillers.popleft()
                    u()
                    debt -= cost
                return debt

            def emit_scores_exp(qc, pair, skc):
                sc = spsum.tile([128, 1024], f32, tag="sc")
                for hh in range(2):
                    nc.tensor.matmul(
                        sc[:, hh * 512 : hh * 512 + 512],
                        lhsT=khT_sb[
                            hh * 64 : hh * 64 + 64,
                            pair,
                            skc * 128 : skc * 128 + 128,
                        ],
                        rhs=qhT_sb[
                            hh * 64 : hh * 64 + 64,
                            pair,
                            qc * 512 : qc * 512 + 512,
                        ],
                        start=True,
                        stop=True,
                    )
                probs = probpool.tile([128, 1024], BF, tag="probs")
                nc.scalar.activation(
                    out=probs,
                    in_=sc,
                    func=Exp,
                    bias=mb_sb[:, skc : skc + 1],
                    scale=0.125,
                )
                return probs

            def emit_attn(pair, skc, probs, ats):
                for hh in range(2):
                    nc.tensor.matmul(
                        ats[hh][0:65, :],
                        lhsT=vh_sb[:, skc, pair * 2 + hh, :],
                        rhs=probs[:, hh * 512 : hh * 512 + 512],
                        start=(skc == 0),
                        stop=(skc == NSK - 1),
                    )

            def make_norm(qc, pair, ats):
                def run():
                    # normalize: replicate denom, approx-reciprocal, multiply
                    rep = spsum.tile([128, 1024], f32, tag="sc")
                    for hh in range(2):
                        dn = smallpool.tile([1, 512], R, tag="dn")
                        nc.vector.tensor_copy(out=dn, in_=ats[hh][64:65, :])
                        nc.tensor.matmul(
                            rep[0:64, hh * 512 : hh * 512 + 512],
                            lhsT=onesr_sb,
                            rhs=dn,
                            start=True,
                            stop=True,
                        )
                        rc = smallpool.tile([64, 512], f32, tag="rc")
                        nc.vector.reciprocal_approx_fast(
                            out=rc, in_=rep[0:64, hh * 512 : hh * 512 + 512]
                        )
                        nc.vector.tensor_mul(
                            concatT_sb[
                                hh * 64 : hh * 64 + 64,
                                pair,
                                qc * 512 : qc * 512 + 512,
                            ],
                            ats[hh][0:64, :],
                            rc,
                        )

                return run

            debt = 0.0
            pending_norm = None
            for qc in range(NQB):
                if qc + 1 < NQB:
                    fillers.extend(
                        proj_block_units(
                            "q", q_d, wq_sb, bq_sb, (qc + 1) * 512, 512, False
                        )
                    )
                for pair in range(NPAIR):
                    # scores run one sk-step ahead of attn, and the previous
                    # pair's normalize is deferred past this pair's first two
                    # score/exp steps, so the scalar engine never idles and
                    # woven fillers never delay the next exp's input.
                    probs_prev = emit_scores_exp(qc, pair, 0)
                    probs_cur = emit_scores_exp(qc, pair, 1)
                    if pending_norm is not None:
                        pending_norm()
                        pending_norm = None
                    at0 = spsum.tile([128, 512], f32, tag="at")
                    at1 = spsum.tile([128, 512], f32, tag="at")
                    ats = (at0, at1)
                    emit_attn(pair, 0, probs_prev, ats)
                    debt = weave(debt + 0.9)
                    for skc in range(2, NSK):
                        probs_next = emit_scores_exp(qc, pair, skc)
                        emit_attn(pair, skc - 1, probs_cur, ats)
                        probs_cur = probs_next
                        debt = weave(debt + 0.45)
                    emit_attn(pair, NSK - 1, probs_cur, ats)
                    debt = weave(debt + 0.45)
                    pending_norm = make_norm(qc, pair, ats)
                fillers.extend(phasec_units(qc))
            pending_norm()
            # drain remaining fillers (last out-projection block)
            debt = weave(1e9)

    nc.compile()
    return nc


def get_nc(SK=SK_FAST):
    if SK not in _cache:
        _cache[SK] = _build_nc(SK)
    return _cache[SK]


def make_in_maps(q, k, v, mask, Wq, bq, Wk, bk, Wv, bv, Wo, bo):
    import ml_dtypes

    bf16 = ml_dtypes.bfloat16
    f32 = np.float32
    c = np.ascontiguousarray

    counts = [int(np.asarray(mask[b, 0]).sum()) for b in range(B)]
    SK = SK_FAST if max(counts) <= SK_FAST else S
    NSK = SK // 128

    grid = np.arange(128)[:, None] + 128 * np.arange(NSK)[None, :]  # [128,NSK]
    per_batch = []
    for b in range(B):
        idx = np.flatnonzero(np.asarray(mask[b, 0]))
        nk = len(idx)
        kc = np.zeros((SK, D), bf16)
        kc[:nk] = np.asarray(k[b], f32)[idx].astype(bf16)
        vc = np.zeros((SK, D), bf16)
        vc[:nk] = np.asarray(v[b], f32)[idx].astype(bf16)
        mb = np.where(grid < nk, 0.0, -1e9).astype(f32)
        ki = np.broadcast_to(
            (grid < nk).astype(bf16)[:, :, None], (128, NSK, 8)
        ).reshape(128, NSK * 8)
        per_batch.append(
            {
                "q": np.asarray(q[b], f32).astype(bf16),
                "kc": kc,
                "vc": vc,
                "maskbias": mb,
                "keyind": c(ki),
            }
        )

    ones = np.ones((128, 512), bf16)
    onesr = np.ones((1, 64), f32)
    ident = np.eye(128, dtype=bf16)
    in_maps = []
    for core in range(NCORES):
        b, g = core // 2, core % 2
        cols = slice(g * CPG, (g + 1) * CPG)
        m = dict(per_batch[b])
        m.update(
            {
                "wq": np.asarray(Wq[:, cols], f32).astype(bf16),
                "wk": np.asarray(Wk[:, cols], f32).astype(bf16),
                "wv": np.asarray(Wv[:, cols], f32).astype(bf16),
                "wo": np.asarray(Wo[cols, :], f32).astype(bf16),
                "bq": np.asarray(bq[cols], f32).astype(bf16),
                "bk": np.asarray(bk[cols], f32).astype(bf16),
                "bv": np.asarray(bv[cols], f32).astype(bf16),
                "ones": ones,
                "onesr": onesr,
                "ident": ident,
            }
        )
        in_maps.append(m)
    return in_maps, SK


def gather(results, bo):
    out = np.zeros((B, S, D), np.float32)
    for core in range(NCORES):
        b = core // 2
        out[b] += results[core]["out"]
    out += np.asarray(bo, np.float32)[None, None, :]
    return out


def run_on_hw(in_maps, SK=SK_FAST, trace=False, trace_cores=None):
    from concourse.bass_utils import run_bass_kernel_spmd

    nc = get_nc(SK)
    return run_bass_kernel_spmd(
        nc,
        in_maps,
        list(range(NCORES)),
        trace=trace,
        trace_cores=trace_cores,
    )


def kernel(q, k, v, mask, Wq, bq, Wk, bk, Wv, bv, Wo, bo):
    in_maps, SK = make_in_maps(q, k, v, mask, Wq, bq, Wk, bk, Wv, bv, Wo, bo)
    res = run_on_hw(in_maps, SK)
    return gather(res.results, bo)


# revision 18
# speedup vs baseline: 1.2638x; 1.0107x over previous
"""Multi-head attention (B=4, S=2048, D=1024, H=16, Dk=64) on 8 trn2 NeuronCores.

Sharding: core = (batch b, head-group g), g selects 8 heads (512 proj cols).
Host sums the two partial out-projections per batch and adds bo.

Key optimizations over the v1 kernel (912us):
  * Host-side key compaction: masked keys give exactly-zero probs in the
    reference (exp(-1e9/8) underflows), so drop them on the host and pad
    k/v to SK=1152 rows (mask is Bernoulli(0.5), so ~1024 survive; fall
    back to SK=2048 if a batch ever exceeds 1152).  Cuts k/v projections,
    scores, attn and the scalar-engine exp work by ~44%.
  * bf16 matmuls everywhere (host pre-casts inputs/weights): same 1
    col/cycle stream rate as f32r but fast weight loads (FWL), half the
    DMA and SBUF footprint.  fp32 accumulation in PSUM.
  * Row-tiled scores: the K=64 scores matmuls of the two heads of a pair
    run concurrently in PE row groups 0/64 (tile_position auto-derived
    from the partition bases) -> 2x PE throughput on scores.
  * Pad-key handling via a per-chunk -1e9 activation bias (probs of pad
    keys are exactly 0), plus an indicator column in vh giving the
    softmax denominator for free (row 64 of the attn accumulator).
  * Normalization via replicate-denominator matmul + reciprocal_approx_fast
    on 64 partitions (the v1 kernel burned 6.5us per [1,1024] serial
    reciprocal).
  * Software-pipelined emission: the q projections for query blocks 1-3
    and the out-projection (phase C) are woven into the ACT-bound
    attention loop as PE filler so the tensor engine never idles (keeps
    the HAM clock gate at 2.4 GHz; the v1 kernel sat at 1.2 GHz for
    600us of its runtime).
"""

import sys

sys.path.insert(0, "/opt/trn_rl_repo")

import numpy as np

B, S, D, H, DK = 4, 2048, 1024, 16, 64
CPG = 512          # projection columns per core (8 heads x 64)
NCORES = 8
SK_FAST = 1152     # compacted+padded key rows (multiple of 128)

_cache = {}


def _build_nc(SK):
    import contextlib
    from collections import deque

    import concourse.bass as bass
    import concourse.tile as tile
    from concourse import bacc, mybir

    f32 = mybir.dt.float32
    R = mybir.dt.float32r
    BF = mybir.dt.bfloat16
    Exp = mybir.ActivationFunctionType.Exp

    NSK = SK // 128        # key chunks of 128
    NQB = S // 512         # query 512-blocks (4)
    NDCH = D // 128        # contraction chunks for projections (8)
    NPAIR = 4              # head pairs per core

    nc = bacc.Bacc("TRN2", target_bir_lowering=False, debug=False)

    q_d = nc.dram_tensor("q", [S, D], BF, kind="ExternalInput").ap()
    k_d = nc.dram_tensor("kc", [SK, D], BF, kind="ExternalInput").ap()
    v_d = nc.dram_tensor("vc", [SK, D], BF, kind="ExternalInput").ap()
    wq_d = nc.dram_tensor("wq", [D, CPG], BF, kind="ExternalInput").ap()
    wk_d = nc.dram_tensor("wk", [D, CPG], BF, kind="ExternalInput").ap()
    wv_d = nc.dram_tensor("wv", [D, CPG], BF, kind="ExternalInput").ap()
    wo_d = nc.dram_tensor("wo", [CPG, D], BF, kind="ExternalInput").ap()
    bq_d = nc.dram_tensor("bq", [CPG], BF, kind="ExternalInput").ap()
    bk_d = nc.dram_tensor("bk", [CPG], BF, kind="ExternalInput").ap()
    bv_d = nc.dram_tensor("bv", [CPG], BF, kind="ExternalInput").ap()
    mb_d = nc.dram_tensor("maskbias", [128, NSK], f32, kind="ExternalInput").ap()
    ki_d = nc.dram_tensor("keyind", [128, NSK * 8], BF, kind="ExternalInput").ap()
    ones_d = nc.dram_tensor("ones", [128, 512], BF, kind="ExternalInput").ap()
    onesr_d = nc.dram_tensor("onesr", [1, 64], R, kind="ExternalInput").ap()
    ident_d = nc.dram_tensor("ident", [128, 128], BF, kind="ExternalInput").ap()
    out_d = nc.dram_tensor("out", [S, D], f32, kind="ExternalOutput").ap()

    with tile.TileContext(nc) as tc:
        import contextlib

        with contextlib.ExitStack() as ctx:
            # ---------- persistent tensors + constants ----------
            persist = ctx.enter_context(tc.tile_pool(name="persist", bufs=1))
            consts = ctx.enter_context(tc.tile_pool(name="consts", bufs=1))

            qhT_sb = persist.tile([128, NPAIR, S], BF)     # [c%128, pair, sq]
            khT_sb = persist.tile([128, NPAIR, SK], BF)
            vh_sb = persist.tile([128, NSK, 8, DK + 1], BF)  # ind col at 64
            concatT_sb = persist.tile([128, NPAIR, S], BF)
            wq_sb = persist.tile([128, NDCH, CPG], BF)
            wk_sb = persist.tile([128, NDCH, CPG], BF)
            wv_sb = persist.tile([128, NDCH, CPG], BF)
            wo_sb = persist.tile([128, NPAIR, D], BF)

            ones_sb = consts.tile([1, 512], BF)
            onesr_sb = consts.tile([1, 64], R)
            ident = consts.tile([128, 128], BF)
            mb_sb = consts.tile([128, NSK], f32)
            bq_sb = consts.tile([1, CPG], BF)
            bk_sb = consts.tile([1, CPG], BF)
            bv_sb = consts.tile([1, CPG], BF)

            # DMA emission is ordered so the PE can start within a few us:
            # first k x-tiles + what the first transposes/projections need;
            # the rest of the weights just-in-time before their consumers.
            def dma_weights(t_sb, t_d):
                for j in range(NDCH):
                    nc.sync.dma_start(
                        out=t_sb[:, j, :], in_=t_d[j * 128 : j * 128 + 128, :]
                    )

            # ---------- shared SBUF rings ----------
            natpool = ctx.enter_context(tc.tile_pool(name="natpool", bufs=8))
            xtpool = ctx.enter_context(tc.tile_pool(name="xtpool", bufs=10))
            probpool = ctx.enter_context(tc.tile_pool(name="probpool", bufs=3))
            smallpool = ctx.enter_context(tc.tile_pool(name="smallpool", bufs=4))
            outpool = ctx.enter_context(tc.tile_pool(name="outpool", bufs=3))

            # PSUM pools are phase-local: the prefix gets deep tp/pr rings
            # (6 banks); the spine re-uses those banks as
            # sc 2x[128,1024]f32 (4) + at 2x[128,512]f32 (2) +
            # fill 2x[128,512] (2) = 8 banks exactly.
            pools = {}

            # ---------- projection block emitters ----------
            def proj_block_units(kind, x_d, w_sb, b_sb, s0, w, act_copy,
                                 nats_in=None):
                """Generate unit-closures for projecting x rows [s0, s0+w).
                kind: 'q'/'k' -> [c, s] into qhT_sb/khT_sb; 'v' -> vh_sb."""
                nsub = w // 128
                nats = nats_in if nats_in is not None else []
                xts = []

                def u_load():
                    for i in range(nsub):
                        nat = natpool.tile([128, D], BF, tag="nat")
                        r0 = s0 + i * 128
                        nc.sync.dma_start(out=nat, in_=x_d[r0 : r0 + 128, :])
                        nats.append(nat)

                if nats_in is None:
                    yield 0.1, u_load

                def u_tp(j):
                    def run():
                        tp = pools["tp"]()
                        for i in range(nsub):
                            nc.tensor.transpose(
                                out=tp[:, i * 128 : i * 128 + 128],
                                in_=nats[i][:, j * 128 : j * 128 + 128],
                                identity=ident,
                            )
                        xt = xtpool.tile([128, 512], BF, tag="xt")
                        if act_copy:
                            nc.scalar.copy(out=xt[:, :w], in_=tp[:, :w])
                        else:
                            nc.vector.tensor_copy(out=xt[:, :w], in_=tp[:, :w])
                        xts.append(xt)

                    return run

                for j in range(NDCH):
                    yield 0.3, u_tp(j)

                if kind in ("q", "k"):
                    dst = qhT_sb if kind == "q" else khT_sb

                    def u_proj(cch, box, j0, j1):
                        def run():
                            if j0 == 0:
                                pr = pools["pr"]()
                                box.append(pr)
                                nc.tensor.matmul(
                                    pr[:, :w],
                                    lhsT=b_sb[0:1, cch * 128 : cch * 128 + 128],
                                    rhs=ones_sb[0:1, :w],
                                    start=True,
                                    stop=False,
                                )
                            pr = box[0]
                            for j in range(j0, j1):
                                nc.tensor.matmul(
                                    pr[:, :w],
                                    lhsT=w_sb[:, j, cch * 128 : cch * 128 + 128],
                                    rhs=xts[j][:, :w],
                                    start=False,
                                    stop=(j == NDCH - 1),
                                )
                            if j1 == NDCH:
                                nc.vector.tensor_copy(
                                    out=dst[:, cch, s0 : s0 + w], in_=pr[:, :w]
                                )

                        return run

                    for cch in range(NPAIR):
                        box = []
                        for j0 in range(0, NDCH, 2):
                            yield 0.5, u_proj(cch, box, j0, j0 + 2)
                else:

                    def u_projv(sub, box, j0, j1):
                        def run():
                            if j0 == 0:
                                pr = pools["pr"]()
                                box.append(pr)
                                nc.tensor.matmul(
                                    pr,
                                    lhsT=ones_sb[0:1, 0:128],
                                    rhs=b_sb[0:1, :],
                                    start=True,
                                    stop=False,
                                )
                            pr = box[0]
                            for j in range(j0, j1):
                                nc.tensor.matmul(
                                    pr,
                                    lhsT=xts[j][:, sub * 128 : sub * 128 + 128],
                                    rhs=w_sb[:, j, :],
                                    start=False,
                                    stop=(j == NDCH - 1),
                                )
                            if j1 == NDCH:
                                skc = (s0 + sub * 128) // 128
                                nc.vector.tensor_copy(
                                    out=vh_sb[:, skc, :, 0:DK],
                                    in_=pr.rearrange("p (h d) -> p h d", h=8),
                                )

                        return run

                    for sub in range(nsub):
                        box = []
                        for j0 in range(0, NDCH, 2):
                            yield 0.5, u_projv(sub, box, j0, j0 + 2)

            def phasec_units(qc):
                """Out-projection for query block qc (concatT -> out)."""

                def u_cblk(sqc, do, half, box):
                    def run():
                        if half == 0:
                            box.append(spsum.tile([128, 512], f32, tag="fill",
                                                  name="o_ps"))
                        o_ps = box[0]
                        for p in (0, 1) if half == 0 else (2, 3):
                            nc.tensor.matmul(
                                o_ps,
                                lhsT=concatT_sb[
                                    :, p, sqc * 128 : sqc * 128 + 128
                                ],
                                rhs=wo_sb[:, p, do * 512 : do * 512 + 512],
                                start=(p == 0),
                                stop=(p == NPAIR - 1),
                            )
                        if half == 1:
                            o_sb = outpool.tile([128, 512], f32, tag="osb")
                            nc.vector.tensor_copy(out=o_sb, in_=o_ps)
                            nc.sync.dma_start(
                                out=out_d[
                                    sqc * 128 : sqc * 128 + 128,
                                    do * 512 : do * 512 + 512,
                                ],
                                in_=o_sb,
                            )

                    return run

                for sq in range(4):
                    for do in range(2):
                        box = []
                        for half in range(2):
                            yield 0.5, u_cblk(qc * 4 + sq, do, half, box)

            def drain(units):
                for _, u in units:
                    u()

            # ---------- prefix: k, v and q block 0 (ACT does the copies) ----
            prefix_ctx = contextlib.ExitStack()
            ppsum = prefix_ctx.enter_context(
                tc.tile_pool(name="ppsum", bufs=2, space="PSUM")
            )
            pools["tp"] = lambda: ppsum.tile([128, 512], BF, tag="tp", bufs=2, name="tp")
            pools["pr"] = lambda: ppsum.tile([128, 512], f32, tag="pr", bufs=4, name="pr")
            kblocks = []
            o = 0
            while o < SK:
                w = min(512, SK - o)
                kblocks.append((o, w))
                o += w

            # first k x-tiles + immediate deps, then weights just-in-time
            knats0 = []
            for i in range(4):
                nat = natpool.tile([128, D], BF, tag="nat")
                nc.sync.dma_start(out=nat, in_=k_d[i * 128 : i * 128 + 128, :])
                knats0.append(nat)
            nc.sync.dma_start(out=ident, in_=ident_d)
            nc.sync.dma_start(out=ones_sb, in_=ones_d[0:1, :])
            nc.sync.dma_start(out=bk_sb, in_=bk_d[None, :])
            dma_weights(wk_sb, wk_d)
            nc.sync.dma_start(out=mb_sb, in_=mb_d)
            nc.sync.dma_start(out=onesr_sb, in_=onesr_d)
            nc.sync.dma_start(
                out=vh_sb[:, :, :, DK],
                in_=ki_d.rearrange("p (a b) -> p a b", a=NSK),
            )
            for j in range(NPAIR):
                nc.sync.dma_start(
                    out=wo_sb[:, j, :], in_=wo_d[j * 128 : j * 128 + 128, :]
                )
            for s0, w in kblocks:
                drain(
                    proj_block_units(
                        "k", k_d, wk_sb, bk_sb, s0, w, True,
                        nats_in=knats0 if s0 == 0 else None,
                    )
                )
            nc.sync.dma_start(out=bv_sb, in_=bv_d[None, :])
            dma_weights(wv_sb, wv_d)
            for s0, w in kblocks:
                drain(proj_block_units("v", v_d, wv_sb, bv_sb, s0, w, True))
            nc.sync.dma_start(out=bq_sb, in_=bq_d[None, :])
            dma_weights(wq_sb, wq_d)
            drain(proj_block_units("q", q_d, wq_sb, bq_sb, 0, 512, True))

            # ---------- attention spine with woven fillers ----------
            prefix_ctx.close()
            spsum = ctx.enter_context(
                tc.tile_pool(name="spsum", bufs=2, space="PSUM")
            )
            pools["tp"] = lambda: spsum.tile([128, 512], BF, tag="fill", name="tpw")
            pools["pr"] = lambda: spsum.tile([128, 512], f32, tag="fill", name="prw")
            fillers = deque()

            def weave(debt):
                while fillers and debt >= fillers[0][0]:
                    cost, u = fillers.popleft()
                    u()
                    debt -= cost
                return debt

            def emit_scores_exp(qc, pair, skc):
                sc = spsum.tile([128, 1024], f32, tag="sc")
                for hh in range(2):
                    nc.tensor.matmul(
                        sc[:, hh * 512 : hh * 512 + 512],
                        lhsT=khT_sb[
                            hh * 64 : hh * 64 + 64,
                            pair,
                            skc * 128 : skc * 128 + 128,
                        ],
                        rhs=qhT_sb[
                            hh * 64 : hh * 64 + 64,
                            pair,
                            qc * 512 : qc * 512 + 512,
                        ],
                        start=True,
                        stop=True,
                    )
                probs = probpool.tile([128, 1024], BF, tag="probs")
                nc.scalar.activation(
                    out=probs,
                    in_=sc,
                    func=Exp,
                    bias=mb_sb[:, skc : skc + 1],
                    scale=0.125,
                )
                return probs

            def emit_attn(pair, skc, probs, ats):
                for hh in range(2):
                    nc.tensor.matmul(
                        ats[hh][0:65, :],
                        lhsT=vh_sb[:, skc, pair * 2 + hh, :],
                        rhs=probs[:, hh * 512 : hh * 512 + 512],
                        start=(skc == 0),
                        stop=(skc == NSK - 1),
                    )

            def make_norm(qc, pair, ats):
                def run():
                    # normalize: replicate denom, approx-reciprocal, multiply
                    rep = spsum.tile([128, 1024], f32, tag="sc")
                    dns = []
                    for hh in range(2):
                        dn = smallpool.tile([1, 512], R, tag="dn")
                        nc.vector.tensor_copy(out=dn, in_=ats[hh][64:65, :])
                        dns.append(dn)
                    for hh in range(2):
                        nc.tensor.matmul(
                            rep[0:64, hh * 512 : hh * 512 + 512],
                            lhsT=onesr_sb,
                            rhs=dns[hh],
                            start=True,
                            stop=True,
                        )
                    rcs = []
                    for hh in range(2):
                        rc = smallpool.tile([64, 512], f32, tag="rc")
                        nc.vector.reciprocal_approx_fast(
                            out=rc, in_=rep[0:64, hh * 512 : hh * 512 + 512]
                        )
                        rcs.append(rc)
                    for hh in range(2):
                        nc.vector.tensor_mul(
                            concatT_sb[
                                hh * 64 : hh * 64 + 64,
                                pair,
                                qc * 512 : qc * 512 + 512,
                            ],
                            ats[hh][0:64, :],
                            rcs[hh],
                        )

                return run

            # ring-parity shim: pairs alloc 10 sc-tag tiles (9 scores + 1
            # deferred rep) but the first pair has no pending rep; without
            # this dummy every pair's first scores wait one exp too many.
            scdum = spsum.tile([128, 1024], f32, tag="sc", name="scdum")
            nc.vector.tensor_copy(out=scdum[0:1, 0:64], in_=onesr_sb)
            dumrd = smallpool.tile([1, 64], f32, tag="dumrd")
            nc.vector.tensor_copy(out=dumrd, in_=scdum[0:1, 0:64])

            debt = 0.0
            pending_norm = None
            for qc in range(NQB):
                if qc + 1 < NQB:
                    fillers.extend(
                        proj_block_units(
                            "q", q_d, wq_sb, bq_sb, (qc + 1) * 512, 512, False
                        )
                    )
                for pair in range(NPAIR):
                    # scores run one sk-step ahead of attn, and the previous
                    # pair's normalize is deferred past this pair's first two
                    # score/exp steps, so the scalar engine never idles and
                    # woven fillers never delay the next exp's input.
                    probs_prev = emit_scores_exp(qc, pair, 0)
                    probs_cur = emit_scores_exp(qc, pair, 1)
                    if pending_norm is not None:
                        pending_norm()
                        pending_norm = None
                    at0 = spsum.tile([128, 512], f32, tag="at")
                    at1 = spsum.tile([128, 512], f32, tag="at")
                    ats = (at0, at1)
                    emit_attn(pair, 0, probs_prev, ats)
                    debt = weave(debt + 0.9)
                    for skc in range(2, NSK):
                        probs_next = emit_scores_exp(qc, pair, skc)
                        emit_attn(pair, skc - 1, probs_cur, ats)
                        probs_cur = probs_next
                        debt = weave(debt + 0.45)
                    emit_attn(pair, NSK - 1, probs_cur, ats)
                    debt = weave(debt + 0.45)
                    pending_norm = make_norm(qc, pair, ats)
                fillers.extend(phasec_units(qc))
            pending_norm()
            # drain remaining fillers (last out-projection block)
            debt = weave(1e9)

    nc.compile()
    return nc


def get_nc(SK=SK_FAST):
    if SK not in _cache:
        _cache[SK] = _build_nc(SK)
    return _cache[SK]


def make_in_maps(q, k, v, mask, Wq, bq, Wk, bk, Wv, bv, Wo, bo):
    import ml_dtypes

    bf16 = ml_dtypes.bfloat16
    f32 = np.float32
    c = np.ascontiguousarray

    counts = [int(np.asarray(mask[b, 0]).sum()) for b in range(B)]
    SK = SK_FAST if max(counts) <= SK_FAST else S
    NSK = SK // 128

    grid = np.arange(128)[:, None] + 128 * np.arange(NSK)[None, :]  # [128,NSK]
    per_batch = []
    for b in range(B):
        idx = np.flatnonzero(np.asarray(mask[b, 0]))
        nk = len(idx)
        kc = np.zeros((SK, D), bf16)
        kc[:nk] = np.asarray(k[b], f32)[idx].astype(bf16)
        vc = np.zeros((SK, D), bf16)
        vc[:nk] = np.asarray(v[b], f32)[idx].astype(bf16)
        mb = np.where(grid < nk, 0.0, -1e9).astype(f32)
        ki = np.broadcast_to(
            (grid < nk).astype(bf16)[:, :, None], (128, NSK, 8)
        ).reshape(128, NSK * 8)
        per_batch.append(
            {
                "q": np.asarray(q[b], f32).astype(bf16),
                "kc": kc,
                "vc": vc,
                "maskbias": mb,
                "keyind": c(ki),
            }
        )

    ones = np.ones((128, 512), bf16)
    onesr = np.ones((1, 64), f32)
    ident = np.eye(128, dtype=bf16)
    in_maps = []
    for core in range(NCORES):
        b, g = core // 2, core % 2
        cols = slice(g * CPG, (g + 1) * CPG)
        m = dict(per_batch[b])
        m.update(
            {
                "wq": np.asarray(Wq[:, cols], f32).astype(bf16),
                "wk": np.asarray(Wk[:, cols], f32).astype(bf16),
                "wv": np.asarray(Wv[:, cols], f32).astype(bf16),
                "wo": np.asarray(Wo[cols, :], f32).astype(bf16),
                "bq": np.asarray(bq[cols], f32).astype(bf16),
                "bk": np.asarray(bk[cols], f32).astype(bf16),
                "bv": np.asarray(bv[cols], f32).astype(bf16),
                "ones": ones,
                "onesr": onesr,
                "ident": ident,
            }
        )
        in_maps.append(m)
    return in_maps, SK


def gather(results, bo):
    out = np.zeros((B, S, D), np.float32)
    for core in range(NCORES):
        b = core // 2
        out[b] += results[core]["out"]
    out += np.asarray(bo, np.float32)[None, None, :]
    return out


def run_on_hw(in_maps, SK=SK_FAST, trace=False, trace_cores=None):
    from concourse.bass_utils import run_bass_kernel_spmd

    nc = get_nc(SK)
    return run_bass_kernel_spmd(
        nc,
        in_maps,
        list(range(NCORES)),
        trace=trace,
        trace_cores=trace_cores,
    )


def kernel(q, k, v, mask, Wq, bq, Wk, bk, Wv, bv, Wo, bo):
    in_maps, SK = make_in_maps(q, k, v, mask, Wq, bq, Wk, bk, Wv, bv, Wo, bo)
    res = run_on_hw(in_maps, SK)
    return gather(res.results, bo)


# revision 19
# speedup vs baseline: 1.2712x; 1.0058x over previous
"""Multi-head attention (B=4, S=2048, D=1024, H=16, Dk=64) on 8 trn2 NeuronCores.

Sharding: core = (batch b, head-group g), g selects 8 heads (512 proj cols).
Host sums the two partial out-projections per batch and adds bo.

Key optimizations over the v1 kernel (912us):
  * Host-side key compaction: masked keys give exactly-zero probs in the
    reference (exp(-1e9/8) underflows), so drop them on the host and pad
    k/v to SK=1152 rows (mask is Bernoulli(0.5), so ~1024 survive; fall
    back to SK=2048 if a batch ever exceeds 1152).  Cuts k/v projections,
    scores, attn and the scalar-engine exp work by ~44%.
  * bf16 matmuls everywhere (host pre-casts inputs/weights): same 1
    col/cycle stream rate as f32r but fast weight loads (FWL), half the
    DMA and SBUF footprint.  fp32 accumulation in PSUM.
  * Row-tiled scores: the K=64 scores matmuls of the two heads of a pair
    run concurrently in PE row groups 0/64 (tile_position auto-derived
    from the partition bases) -> 2x PE throughput on scores.
  * Pad-key handling via a per-chunk -1e9 activation bias (probs of pad
    keys are exactly 0), plus an indicator column in vh giving the
    softmax denominator for free (row 64 of the attn accumulator).
  * Normalization via replicate-denominator matmul + reciprocal_approx_fast
    on 64 partitions (the v1 kernel burned 6.5us per [1,1024] serial
    reciprocal).
  * Software-pipelined emission: the q projections for query blocks 1-3
    and the out-projection (phase C) are woven into the ACT-bound
    attention loop as PE filler so the tensor engine never idles (keeps
    the HAM clock gate at 2.4 GHz; the v1 kernel sat at 1.2 GHz for
    600us of its runtime).
"""

import sys

sys.path.insert(0, "/opt/trn_rl_repo")

import numpy as np

B, S, D, H, DK = 4, 2048, 1024, 16, 64
CPG = 512          # projection columns per core (8 heads x 64)
NCORES = 8
SK_FAST = 1152     # compacted+padded key rows (multiple of 128)

_cache = {}


def _build_nc(SK):
    import contextlib
    from collections import deque

    import concourse.bass as bass
    import concourse.tile as tile
    from concourse import bacc, mybir

    f32 = mybir.dt.float32
    R = mybir.dt.float32r
    BF = mybir.dt.bfloat16
    Exp = mybir.ActivationFunctionType.Exp

    NSK = SK // 128        # key chunks of 128
    NQB = S // 512         # query 512-blocks (4)
    NDCH = D // 128        # contraction chunks for projections (8)
    NPAIR = 4              # head pairs per core

    nc = bacc.Bacc("TRN2", target_bir_lowering=False, debug=False)

    q_d = nc.dram_tensor("q", [S, D], BF, kind="ExternalInput").ap()
    k_d = nc.dram_tensor("kc", [SK, D], BF, kind="ExternalInput").ap()
    v_d = nc.dram_tensor("vc", [SK, D], BF, kind="ExternalInput").ap()
    wq_d = nc.dram_tensor("wq", [D, CPG], BF, kind="ExternalInput").ap()
    wk_d = nc.dram_tensor("wk", [D, CPG], BF, kind="ExternalInput").ap()
    wv_d = nc.dram_tensor("wv", [D, CPG], BF, kind="ExternalInput").ap()
    wo_d = nc.dram_tensor("wo", [CPG, D], BF, kind="ExternalInput").ap()
    bq_d = nc.dram_tensor("bq", [CPG], BF, kind="ExternalInput").ap()
    bk_d = nc.dram_tensor("bk", [CPG], BF, kind="ExternalInput").ap()
    bv_d = nc.dram_tensor("bv", [CPG], BF, kind="ExternalInput").ap()
    mb_d = nc.dram_tensor("maskbias", [128, NSK], f32, kind="ExternalInput").ap()
    ki_d = nc.dram_tensor("keyind", [128, NSK * 8], BF, kind="ExternalInput").ap()
    ones_d = nc.dram_tensor("ones", [128, 512], BF, kind="ExternalInput").ap()
    onesr_d = nc.dram_tensor("onesr", [1, 64], R, kind="ExternalInput").ap()
    ident_d = nc.dram_tensor("ident", [128, 128], BF, kind="ExternalInput").ap()
    out_d = nc.dram_tensor("out", [S, D], f32, kind="ExternalOutput").ap()

    with tile.TileContext(nc) as tc:
        import contextlib

        with contextlib.ExitStack() as ctx:
            # ---------- persistent tensors + constants ----------
            persist = ctx.enter_context(tc.tile_pool(name="persist", bufs=1))
            consts = ctx.enter_context(tc.tile_pool(name="consts", bufs=1))

            qhT_sb = persist.tile([128, NPAIR, S], BF)     # [c%128, pair, sq]
            khT_sb = persist.tile([128, NPAIR, SK], BF)
            vh_sb = persist.tile([128, NSK, 8, DK + 1], BF)  # ind col at 64
            concatT_sb = persist.tile([128, NPAIR, S], BF)
            wq_sb = persist.tile([128, NDCH, CPG], BF)
            wk_sb = persist.tile([128, NDCH, CPG], BF)
            wv_sb = persist.tile([128, NDCH, CPG], BF)
            wo_sb = persist.tile([128, NPAIR, D], BF)

            ones_sb = consts.tile([1, 512], BF)
            onesr_sb = consts.tile([1, 64], R)
            ident = consts.tile([128, 128], BF)
            mb_sb = consts.tile([128, NSK], f32)
            bq_sb = consts.tile([1, CPG], BF)
            bk_sb = consts.tile([1, CPG], BF)
            bv_sb = consts.tile([1, CPG], BF)

            # DMA emission is ordered so the PE can start within a few us:
            # first k x-tiles + what the first transposes/projections need;
            # the rest of the weights just-in-time before their consumers.
            def dma_weights(t_sb, t_d):
                for j in range(NDCH):
                    nc.sync.dma_start(
                        out=t_sb[:, j, :], in_=t_d[j * 128 : j * 128 + 128, :]
                    )

            # ---------- shared SBUF rings ----------
            natpool = ctx.enter_context(tc.tile_pool(name="natpool", bufs=8))
            xtpool = ctx.enter_context(tc.tile_pool(name="xtpool", bufs=10))
            probpool = ctx.enter_context(tc.tile_pool(name="probpool", bufs=3))
            smallpool = ctx.enter_context(tc.tile_pool(name="smallpool", bufs=4))
            outpool = ctx.enter_context(tc.tile_pool(name="outpool", bufs=3))

            # PSUM pools are phase-local: the prefix gets deep tp/pr rings
            # (6 banks); the spine re-uses those banks as
            # sc 2x[128,1024]f32 (4) + at 2x[128,512]f32 (2) +
            # fill 2x[128,512] (2) = 8 banks exactly.
            pools = {}

            # ---------- projection block emitters ----------
            def proj_block_units(kind, x_d, w_sb, b_sb, s0, w, act_copy,
                                 nats_in=None):
                """Generate unit-closures for projecting x rows [s0, s0+w).
                kind: 'q'/'k' -> [c, s] into qhT_sb/khT_sb; 'v' -> vh_sb."""
                nsub = w // 128
                nats = nats_in if nats_in is not None else []
                xts = []

                def u_load():
                    for i in range(nsub):
                        nat = natpool.tile([128, D], BF, tag="nat")
                        r0 = s0 + i * 128
                        nc.sync.dma_start(out=nat, in_=x_d[r0 : r0 + 128, :])
                        nats.append(nat)

                if nats_in is None:
                    yield 0.1, u_load

                def u_tp(j):
                    def run():
                        tp = pools["tp"]()
                        for i in range(nsub):
                            nc.tensor.transpose(
                                out=tp[:, i * 128 : i * 128 + 128],
                                in_=nats[i][:, j * 128 : j * 128 + 128],
                                identity=ident,
                            )
                        xt = xtpool.tile([128, 512], BF, tag="xt")
                        if act_copy:
                            nc.scalar.copy(out=xt[:, :w], in_=tp[:, :w])
                        else:
                            nc.vector.tensor_copy(out=xt[:, :w], in_=tp[:, :w])
                        xts.append(xt)

                    return run

                for j in range(NDCH):
                    yield 0.3, u_tp(j)

                if kind in ("q", "k"):
                    dst = qhT_sb if kind == "q" else khT_sb

                    def u_proj(cch, box, j0, j1):
                        def run():
                            if j0 == 0:
                                pr = pools["pr"]()
                                box.append(pr)
                                nc.tensor.matmul(
                                    pr[:, :w],
                                    lhsT=b_sb[0:1, cch * 128 : cch * 128 + 128],
                                    rhs=ones_sb[0:1, :w],
                                    start=True,
                                    stop=False,
                                )
                            pr = box[0]
                            for j in range(j0, j1):
                                nc.tensor.matmul(
                                    pr[:, :w],
                                    lhsT=w_sb[:, j, cch * 128 : cch * 128 + 128],
                                    rhs=xts[j][:, :w],
                                    start=False,
                                    stop=(j == NDCH - 1),
                                )
                            if j1 == NDCH:
                                nc.vector.tensor_copy(
                                    out=dst[:, cch, s0 : s0 + w], in_=pr[:, :w]
                                )

                        return run

                    for cch in range(NPAIR):
                        box = []
                        for j0 in range(0, NDCH, 2):
                            yield 0.5, u_proj(cch, box, j0, j0 + 2)
                else:

                    def u_projv(sub, box, j0, j1):
                        def run():
                            if j0 == 0:
                                pr = pools["pr"]()
                                box.append(pr)
                                nc.tensor.matmul(
                                    pr,
                                    lhsT=ones_sb[0:1, 0:128],
                                    rhs=b_sb[0:1, :],
                                    start=True,
                                    stop=False,
                                )
                            pr = box[0]
                            for j in range(j0, j1):
                                nc.tensor.matmul(
                                    pr,
                                    lhsT=xts[j][:, sub * 128 : sub * 128 + 128],
                                    rhs=w_sb[:, j, :],
                                    start=False,
                                    stop=(j == NDCH - 1),
                                )
                            if j1 == NDCH:
                                skc = (s0 + sub * 128) // 128
                                nc.vector.tensor_copy(
                                    out=vh_sb[:, skc, :, 0:DK],
                                    in_=pr.rearrange("p (h d) -> p h d", h=8),
                                )

                        return run

                    for sub in range(nsub):
                        box = []
                        for j0 in range(0, NDCH, 2):
                            yield 0.5, u_projv(sub, box, j0, j0 + 2)

            def phasec_units(qc):
                """Out-projection for query block qc (concatT -> out)."""

                def u_cblk(sqc, do, half, box):
                    def run():
                        if half == 0:
                            box.append(spsum.tile([128, 512], f32, tag="fill",
                                                  name="o_ps"))
                        o_ps = box[0]
                        for p in (0, 1) if half == 0 else (2, 3):
                            nc.tensor.matmul(
                                o_ps,
                                lhsT=concatT_sb[
                                    :, p, sqc * 128 : sqc * 128 + 128
                                ],
                                rhs=wo_sb[:, p, do * 512 : do * 512 + 512],
                                start=(p == 0),
                                stop=(p == NPAIR - 1),
                            )
                        if half == 1:
                            o_sb = outpool.tile([128, 512], f32, tag="osb")
                            nc.vector.tensor_copy(out=o_sb, in_=o_ps)
                            nc.sync.dma_start(
                                out=out_d[
                                    sqc * 128 : sqc * 128 + 128,
                                    do * 512 : do * 512 + 512,
                                ],
                                in_=o_sb,
                            )

                    return run

                for sq in range(4):
                    for do in range(2):
                        box = []
                        for half in range(2):
                            yield 0.5, u_cblk(qc * 4 + sq, do, half, box)

            def drain(units):
                for _, u in units:
                    u()

            # ---------- prefix: k, v and q block 0 (ACT does the copies) ----
            prefix_ctx = contextlib.ExitStack()
            ppsum = prefix_ctx.enter_context(
                tc.tile_pool(name="ppsum", bufs=2, space="PSUM")
            )
            pools["tp"] = lambda: ppsum.tile([128, 512], BF, tag="tp", bufs=2, name="tp")
            pools["pr"] = lambda: ppsum.tile([128, 512], f32, tag="pr", bufs=4, name="pr")
            kblocks = []
            o = 0
            while o < SK:
                w = min(512, SK - o)
                kblocks.append((o, w))
                o += w

            # first k x-tiles + immediate deps, then weights just-in-time
            knats0 = []
            for i in range(4):
                nat = natpool.tile([128, D], BF, tag="nat")
                nc.sync.dma_start(out=nat, in_=k_d[i * 128 : i * 128 + 128, :])
                knats0.append(nat)
            nc.sync.dma_start(out=ident, in_=ident_d)
            nc.sync.dma_start(out=ones_sb, in_=ones_d[0:1, :])
            nc.sync.dma_start(out=bk_sb, in_=bk_d[None, :])
            # warm the PE clock gate (HAM) with throwaway matmuls while the
            # first x-tiles and weights are still in flight; ~6us of PE
            # activity lifts the clock from 1.2 to 2.4 GHz before real work
            warm = ppsum.tile([128, 512], f32, tag="pr", bufs=4, name="warm")
            for i in range(16):
                nc.tensor.matmul(
                    warm[0:1, :],
                    lhsT=ones_sb[0:1, 0:1],
                    rhs=ones_sb[0:1, :],
                    start=True,
                    stop=True,
                )
            warmrd = smallpool.tile([1, 64], f32, tag="dumrd")
            nc.vector.tensor_copy(out=warmrd, in_=warm[0:1, 0:64])
            dma_weights(wk_sb, wk_d)
            nc.sync.dma_start(out=mb_sb, in_=mb_d)
            nc.sync.dma_start(out=onesr_sb, in_=onesr_d)
            nc.sync.dma_start(
                out=vh_sb[:, :, :, DK],
                in_=ki_d.rearrange("p (a b) -> p a b", a=NSK),
            )
            for j in range(NPAIR):
                nc.sync.dma_start(
                    out=wo_sb[:, j, :], in_=wo_d[j * 128 : j * 128 + 128, :]
                )
            for s0, w in kblocks:
                drain(
                    proj_block_units(
                        "k", k_d, wk_sb, bk_sb, s0, w, True,
                        nats_in=knats0 if s0 == 0 else None,
                    )
                )
            nc.sync.dma_start(out=bv_sb, in_=bv_d[None, :])
            dma_weights(wv_sb, wv_d)
            for s0, w in kblocks:
                drain(proj_block_units("v", v_d, wv_sb, bv_sb, s0, w, True))
            nc.sync.dma_start(out=bq_sb, in_=bq_d[None, :])
            dma_weights(wq_sb, wq_d)
            drain(proj_block_units("q", q_d, wq_sb, bq_sb, 0, 512, True))

            # ---------- attention spine with woven fillers ----------
            prefix_ctx.close()
            spsum = ctx.enter_context(
                tc.tile_pool(name="spsum", bufs=2, space="PSUM")
            )
            pools["tp"] = lambda: spsum.tile([128, 512], BF, tag="fill", name="tpw")
            pools["pr"] = lambda: spsum.tile([128, 512], f32, tag="fill", name="prw")
            fillers = deque()

            def weave(debt):
                while fillers and debt >= fillers[0][0]:
                    cost, u = fillers.popleft()
                    u()
                    debt -= cost
                return debt

            def emit_scores_exp(qc, pair, skc):
                sc = spsum.tile([128, 1024], f32, tag="sc")
                for hh in range(2):
                    nc.tensor.matmul(
                        sc[:, hh * 512 : hh * 512 + 512],
                        lhsT=khT_sb[
                            hh * 64 : hh * 64 + 64,
                            pair,
                            skc * 128 : skc * 128 + 128,
                        ],
                        rhs=qhT_sb[
                            hh * 64 : hh * 64 + 64,
                            pair,
                            qc * 512 : qc * 512 + 512,
                        ],
                        start=True,
                        stop=True,
                    )
                probs = probpool.tile([128, 1024], BF, tag="probs")
                nc.scalar.activation(
                    out=probs,
                    in_=sc,
                    func=Exp,
                    bias=mb_sb[:, skc : skc + 1],
                    scale=0.125,
                )
                return probs

            def emit_attn(pair, skc, probs, ats):
                for hh in range(2):
                    nc.tensor.matmul(
                        ats[hh][0:65, :],
                        lhsT=vh_sb[:, skc, pair * 2 + hh, :],
                        rhs=probs[:, hh * 512 : hh * 512 + 512],
                        start=(skc == 0),
                        stop=(skc == NSK - 1),
                    )

            def make_norm(qc, pair, ats):
                def run():
                    # normalize: replicate denom, approx-reciprocal, multiply
                    rep = spsum.tile([128, 1024], f32, tag="sc")
                    dns = []
                    for hh in range(2):
                        dn = smallpool.tile([1, 512], R, tag="dn")
                        nc.vector.tensor_copy(out=dn, in_=ats[hh][64:65, :])
                        dns.append(dn)
                    for hh in range(2):
                        nc.tensor.matmul(
                            rep[0:64, hh * 512 : hh * 512 + 512],
                            lhsT=onesr_sb,
                            rhs=dns[hh],
                            start=True,
                            stop=True,
                        )
                    rcs = []
                    for hh in range(2):
                        rc = smallpool.tile([64, 512], f32, tag="rc")
                        nc.vector.reciprocal_approx_fast(
                            out=rc, in_=rep[0:64, hh * 512 : hh * 512 + 512]
                        )
                        rcs.append(rc)
                    for hh in range(2):
                        nc.vector.tensor_mul(
                            concatT_sb[
                                hh * 64 : hh * 64 + 64,
                                pair,
                                qc * 512 : qc * 512 + 512,
                            ],
                            ats[hh][0:64, :],
                            rcs[hh],
                        )

                return run

            # ring-parity shim: pairs alloc 10 sc-tag tiles (9 scores + 1
            # deferred rep) but the first pair has no pending rep; without
            # this dummy every pair's first scores wait one exp too many.
            scdum = spsum.tile([128, 1024], f32, tag="sc", name="scdum")
            nc.vector.tensor_copy(out=scdum[0:1, 0:64], in_=onesr_sb)
            dumrd = smallpool.tile([1, 64], f32, tag="dumrd")
            nc.vector.tensor_copy(out=dumrd, in_=scdum[0:1, 0:64])

            debt = 0.0
            pending_norm = None
            for qc in range(NQB):
                if qc + 1 < NQB:
                    fillers.extend(
                        proj_block_units(
                            "q", q_d, wq_sb, bq_sb, (qc + 1) * 512, 512, False
                        )
                    )
                for pair in range(NPAIR):
                    # scores run one sk-step ahead of attn, and the previous
                    # pair's normalize is deferred past this pair's first two
                    # score/exp steps, so the scalar engine never idles and
                    # woven fillers never delay the next exp's input.
                    probs_prev = emit_scores_exp(qc, pair, 0)
                    probs_cur = emit_scores_exp(qc, pair, 1)
                    if pending_norm is not None:
                        pending_norm()
                        pending_norm = None
                    at0 = spsum.tile([128, 512], f32, tag="at")
                    at1 = spsum.tile([128, 512], f32, tag="at")
                    ats = (at0, at1)
                    emit_attn(pair, 0, probs_prev, ats)
                    debt = weave(debt + 0.9)
                    for skc in range(2, NSK):
                        probs_next = emit_scores_exp(qc, pair, skc)
                        emit_attn(pair, skc - 1, probs_cur, ats)
                        probs_cur = probs_next
                        debt = weave(debt + 0.45)
                    emit_attn(pair, NSK - 1, probs_cur, ats)
                    debt = weave(debt + 0.45)
                    pending_norm = make_norm(qc, pair, ats)
                fillers.extend(phasec_units(qc))
            pending_norm()
            # drain remaining fillers (last out-projection block)
            debt = weave(1e9)

    nc.compile()
    return nc


def get_nc(SK=SK_FAST):
    if SK not in _cache:
        _cache[SK] = _build_nc(SK)
    return _cache[SK]


def make_in_maps(q, k, v, mask, Wq, bq, Wk, bk, Wv, bv, Wo, bo):
    import ml_dtypes

    bf16 = ml_dtypes.bfloat16
    f32 = np.float32
    c = np.ascontiguousarray

    counts = [int(np.asarray(mask[b, 0]).sum()) for b in range(B)]
    SK = SK_FAST if max(counts) <= SK_FAST else S
    NSK = SK // 128

    grid = np.arange(128)[:, None] + 128 * np.arange(NSK)[None, :]  # [128,NSK]
    per_batch = []
    for b in range(B):
        idx = np.flatnonzero(np.asarray(mask[b, 0]))
        nk = len(idx)
        kc = np.zeros((SK, D), bf16)
        kc[:nk] = np.asarray(k[b], f32)[idx].astype(bf16)
        vc = np.zeros((SK, D), bf16)
        vc[:nk] = np.asarray(v[b], f32)[idx].astype(bf16)
        mb = np.where(grid < nk, 0.0, -1e9).astype(f32)
        ki = np.broadcast_to(
            (grid < nk).astype(bf16)[:, :, None], (128, NSK, 8)
        ).reshape(128, NSK * 8)
        per_batch.append(
            {
                "q": np.asarray(q[b], f32).astype(bf16),
                "kc": kc,
                "vc": vc,
                "maskbias": mb,
                "keyind": c(ki),
            }
        )

    ones = np.ones((128, 512), bf16)
    onesr = np.ones((1, 64), f32)
    ident = np.eye(128, dtype=bf16)
    in_maps = []
    for core in range(NCORES):
        b, g = core // 2, core % 2
        cols = slice(g * CPG, (g + 1) * CPG)
        m = dict(per_batch[b])
        m.update(
            {
                "wq": np.asarray(Wq[:, cols], f32).astype(bf16),
                "wk": np.asarray(Wk[:, cols], f32).astype(bf16),
                "wv": np.asarray(Wv[:, cols], f32).astype(bf16),
                "wo": np.asarray(Wo[cols, :], f32).astype(bf16),
                "bq": np.asarray(bq[cols], f32).astype(bf16),
                "bk": np.asarray(bk[cols], f32).astype(bf16),
                "bv": np.asarray(bv[cols], f32).astype(bf16),
                "ones": ones,
                "onesr": onesr,
                "ident": ident,
            }
        )
        in_maps.append(m)
    return in_maps, SK


def gather(results, bo):
    out = np.zeros((B, S, D), np.float32)
    for core in range(NCORES):
        b = core // 2
        out[b] += results[core]["out"]
    out += np.asarray(bo, np.float32)[None, None, :]
    return out


def run_on_hw(in_maps, SK=SK_FAST, trace=False, trace_cores=None):
    from concourse.bass_utils import run_bass_kernel_spmd

    nc = get_nc(SK)
    return run_bass_kernel_spmd(
        nc,
        in_maps,
        list(range(NCORES)),
        trace=trace,
        trace_cores=trace_cores,
    )


def kernel(q, k, v, mask, Wq, bq, Wk, bk, Wv, bv, Wo, bo):
    in_maps, SK = make_in_maps(q, k, v, mask, Wq, bq, Wk, bk, Wv, bv, Wo, bo)
    res = run_on_hw(in_maps, SK)
    return gather(res.results, bo)
